# revision 3
# baseline (speedup 1.0000x reference)
"""Multi-head causal attention (B=2, S=2048, D=1024, H=16, d=64) on 8 trn2 cores.

Sharding: core c -> batch b=c//4, head-group hg=c%4 (4 heads, 256 of 1024 dims).
Each core computes its 4 heads' attention + its partial out-projection; host
sums the 4 partials per batch and adds the bias.

Design (from the bf16 baseline at ~156us to ~145us):
- q/k projections run as fp8e4m3 DoubleRow matmuls (x and Wq/Wk quantized to
  fp8 on the host, ic-chunk pairs packed per instruction) -> half the PE time
  for those matmuls. V projection and out-projection stay bf16 (rehearsed:
  V-path fp8 noise lands directly in the output).
- AV for OFF-DIAGONAL key tiles runs as fp8 DoubleRow over key-tile pairs:
  exp writes P straight to fp8, V is re-quantized to a 96-wide padded fp8
  copy (64 dims + ones column for the softmax denominator + zero pad;
  DoubleRow stationary width must be a multiple of 32). Diagonal tiles stay
  bf16: host rehearsal showed the fp8 error of the whole AV path lives in
  short rows that only touch diagonal blocks, so off-diagonal fp8 is free
  (adds 0.000 to rel err) while diagonal fp8 would blow the error budget.
- Diagonal V tiles are padded to 96 too so every AV matmul in a pass writes
  the same [96, 512] PSUM region (keeps start/stop accumulation groups
  legal); qc==0/1-style passes order units so a full-width matmul opens and
  closes the group.
- Causal mask multiply trimmed to the 128 columns containing the triangle.
- Scheduling: per-pass ordered filler queues move deferrable PE work
  (projections for later chunks, out-projection batches) into the ACT-bound
  late passes; fillers and due AV flushes are emitted BEFORE each unit's
  score matmuls so an exp-gated score never starves ready work behind it.
  Softmax denominator reciprocal runs on [1,512] before the partition
  broadcast; vext8 fp8 re-quant copies are separate fillers scheduled into
  DVE-idle windows.
- DMA: fp8 x/w startup tiles in ic-pair granularity on the sync queue (first
  DoubleRow matmul fires after ~160KB); wv/wo/bf16-x-chunk0 on the scalar
  queue before any exp is queued; later x chunks are issued 1-2 passes ahead
  of first use, all on the sync HWDGE queue (gpsimd-issued DMAs complete
  late; scalar-queue DMAs would block exp).
"""
import sys

sys.path.insert(0, "/opt/trn_rl_repo")

import numpy as np
import ml_dtypes
import concourse.bass as bass
import concourse.mybir as mybir
from concourse import bacc
from concourse.tile import TileContext
from concourse.bass_utils import run_bass_kernel_spmd

F32 = mybir.dt.float32
BF16 = mybir.dt.bfloat16
FP8 = mybir.dt.float8e4
AF = mybir.ActivationFunctionType
OP = mybir.AluOpType
DR = mybir.MatmulPerfMode.DoubleRow

S = 2048          # sequence length
D = 1024          # model dim
HD = 64           # head dim
NHL = 4           # heads per core
DL = 256          # local out dims (NHL * HD)
NQC = 4           # q chunks of 512
QW = 512          # q chunk width
NST = 16          # seq tiles of 128
NIC = 8           # input-dim chunks of 128
NPR = 4           # ic pairs (DoubleRow)
VW = 96           # padded vext width (64 dims + ones + pad)
LAG = 3           # AV trails scores by this many units


def build_bass():
    nc = bacc.Bacc("TRN2", target_bir_lowering=False, debug=False, num_devices=8)

    x8_d = nc.dram_tensor("x8", [D, S], FP8, kind="ExternalInput")
    xt_d = nc.dram_tensor("xt", [D, S], BF16, kind="ExternalInput")
    wq_d = nc.dram_tensor("wq", [D, DL], FP8, kind="ExternalInput")
    wk_d = nc.dram_tensor("wk", [D, DL], FP8, kind="ExternalInput")
    wv_d = nc.dram_tensor("wv", [D, DL], BF16, kind="ExternalInput")
    wo_d = nc.dram_tensor("wo", [DL, D], BF16, kind="ExternalInput")
    mb_d = nc.dram_tensor("maskb", [128, 2, 128], BF16, kind="ExternalInput")
    out_d = nc.dram_tensor("out", [S, D], BF16, kind="ExternalOutput")

    with TileContext(nc) as tc:
        with (
            tc.tile_pool(name="consts", bufs=1) as consts,
            tc.tile_pool(name="xtp", bufs=1) as xtp,
            tc.tile_pool(name="qk", bufs=1) as qkp,
            tc.tile_pool(name="vv", bufs=1) as vvp,
            tc.tile_pool(name="ctxn", bufs=1) as ctxnp,
            tc.tile_pool(name="ptp", bufs=4) as ptp,
            tc.tile_pool(name="pt8p", bufs=4) as pt8p,
            tc.tile_pool(name="recp", bufs=4) as recp,
            tc.tile_pool(name="rbp", bufs=4) as rbp,
            tc.tile_pool(name="outp", bufs=3) as outp,
            tc.tile_pool(name="psA", bufs=2, space="PSUM") as psA,
            tc.tile_pool(name="psS", bufs=2, space="PSUM") as psS,
            tc.tile_pool(name="psC", bufs=2, space="PSUM") as psC,
        ):
            # ---- SBUF tiles
            # fp8 q/k weights: per p-half, per ic-pair tile [128, 2, 128]
            wqh = [
                [consts.tile([128, 2, 128], FP8, tag=f"wq{p}_{r}", name=f"wq{p}_{r}") for r in range(NPR)]
                for p in range(2)
            ]
            wkh = [
                [consts.tile([128, 2, 128], FP8, tag=f"wk{p}_{r}", name=f"wk{p}_{r}") for r in range(NPR)]
                for p in range(2)
            ]
            wv = consts.tile([128, NIC, DL], BF16, tag="wv")
            wo = consts.tile([128, 2, D], BF16, tag="wo")
            maskb = consts.tile([128, 2, 128], BF16, tag="maskb")
            onesr = consts.tile([1, HD], F32, tag="onesr")
            # fp8 x: chunk 0 in ic-pair tiles (startup), chunks 1-3 whole
            x8c0 = [xtp.tile([128, 2, QW], FP8, tag=f"x8c0_{r}", name=f"x8c0_{r}") for r in range(NPR)]
            x8s = [None] + [
                xtp.tile([128, NIC, QW], FP8, tag=f"x8_{c}", name=f"x8_{c}") for c in range(1, NQC)
            ]
            xts = [xtp.tile([128, NIC, QW], BF16, tag=f"xt{c}", name=f"xt{c}") for c in range(NQC)]
            qt = qkp.tile([128, 2, S], BF16, tag="qt")
            kt = qkp.tile([128, 2, S], BF16, tag="kt")
            # bf16 V (diag AV): [128, st, h, 96] (64 dims + ones + zero pad)
            vextb = vvp.tile([128, NST, NHL, VW], BF16, tag="vextb")
            # fp8 V (off-diag DR AV): key-tile pairs [128, m, h, par, 96]
            vext8 = vvp.tile([128, NST // 2, NHL, 2, VW], FP8, tag="vext8")
            ctxn = ctxnp.tile([128, 2, S], BF16, tag="ctxn")

            def x8pair(qc, r):
                return x8c0[r] if qc == 0 else x8s[qc][:, 2 * r : 2 * r + 2, :]

            # ---- DMAs.  Startup order: wq/x8 ic-pairs interleaved on sync so
            # the first DoubleRow matmul fires after ~160KB; wk pairs on scalar.
            def wpair_dma(eng, dst, src_d, p, r):
                eng.dma_start(
                    out=dst,
                    in_=src_d.ap()[256 * r : 256 * (r + 1), 128 * p : 128 * (p + 1)]
                    .rearrange("(c p) n -> p c n", p=128),
                )

            def x8c0_dma(eng, r):
                eng.dma_start(
                    out=x8c0[r],
                    in_=x8_d.ap()[256 * r : 256 * (r + 1), 0:QW].rearrange(
                        "(c p) s -> p c s", p=128
                    ),
                )

            def x8dma(eng, c, lo, hi):
                qsl = slice(c * QW, (c + 1) * QW)
                eng.dma_start(
                    out=x8s[c][:, lo:hi, :],
                    in_=x8_d.ap()[128 * lo : 128 * hi, qsl].rearrange(
                        "(c p) s -> p c s", p=128
                    ),
                )

            def xdma(eng, c, lo, hi):
                qsl = slice(c * QW, (c + 1) * QW)
                eng.dma_start(
                    out=xts[c][:, lo:hi, :],
                    in_=xt_d.ap()[128 * lo : 128 * hi, qsl].rearrange(
                        "(c p) s -> p c s", p=128
                    ),
                )

            for r in range(NPR):
                wpair_dma(nc.sync, wqh[0][r], wq_d, 0, r)
                x8c0_dma(nc.sync, r)
                wpair_dma(nc.scalar, wkh[0][r], wk_d, 0, r)
            xdma(nc.scalar, 0, 0, 4)
            nc.scalar.dma_start(out=wv, in_=wv_d.ap().rearrange("(c p) n -> p c n", p=128))
            xdma(nc.scalar, 0, 4, 8)
            for r in range(NPR):
                wpair_dma(nc.sync, wqh[1][r], wq_d, 1, r)
                wpair_dma(nc.sync, wkh[1][r], wk_d, 1, r)
            nc.scalar.dma_start(
                out=wo, in_=wo_d.ap().rearrange("(c p) n -> p c n", p=128)
            )
            nc.scalar.dma_start(out=maskb, in_=mb_d.ap())
            nc.vector.memset(onesr, 1.0)
            # vext ones column + zero pad (both copies), one-time memsets
            nc.vector.memset(vextb[:, :, :, HD : HD + 1], 1.0)
            nc.vector.memset(vextb[:, :, :, HD + 1 : VW], 0.0)
            nc.gpsimd.memset(vext8[:, :, :, :, HD : HD + 1], 1.0)
            nc.gpsimd.memset(vext8[:, :, :, :, HD + 1 : VW], 0.0)

            # ---- PE filler scheduling: per-pass ordered queues + overflow.
            # Pass order interleaves the ACT-heavy qc=3 passes mid-kernel so
            # deferred projections/out-projections can feed the PE there.
            PASS_ORDER = [(0, 0), (0, 1), (1, 0), (1, 1), (2, 0), (2, 1), (3, 0), (3, 1)]
            fq = {pk: [] for pk in PASS_ORDER}
            overflow = []
            cur_pass = [0]

            def pop_filler(n=1):
                for _ in range(n):
                    q = fq[PASS_ORDER[cur_pass[0]]]
                    if q:
                        q.pop(0)()
                    elif overflow:
                        overflow.pop(0)()
                    else:
                        return

            def drain_before(pi):
                for pk in PASS_ORDER[:pi]:
                    for fn in fq[pk]:
                        fn()
                    fq[pk].clear()
                while overflow:
                    overflow.pop(0)()

            def mk_qtkt(dst, whs, p, qc):
                def go():
                    acc = psA.tile([128, QW], F32, tag="pa")
                    for r in range(NPR):
                        nc.tensor.matmul(
                            acc,
                            whs[p][r],
                            x8pair(qc, r),
                            start=(r == 0),
                            stop=(r == NPR - 1),
                            perf_mode=DR,
                        )
                    nc.vector.tensor_copy(dst[:, p, qc * QW : (qc + 1) * QW], acc)
                return go

            def mk_v(st):
                def go():
                    c, r = divmod(st, 4)
                    acc_t = psA.tile([128, QW], F32, tag="pa")
                    acc = acc_t[:, 0:DL]
                    for ic in range(NIC):
                        nc.tensor.matmul(
                            acc,
                            xts[c][:, ic, 128 * r : 128 * (r + 1)],
                            wv[:, ic, :],
                            start=(ic == 0),
                            stop=(ic == NIC - 1),
                        )
                    nc.vector.tensor_copy(
                        vextb[:, st, :, 0:HD], acc.rearrange("p (h e) -> p h e", h=NHL)
                    )
                return go

            def mk_v8cast(st):
                def go():
                    nc.vector.tensor_copy(
                        vext8[:, st // 2, :, st % 2, 0:HD], vextb[:, st, :, 0:HD]
                    )
                return go

            def mk_outproj(t, tail=False):
                def go():
                    tsl = slice(128 * t, 128 * (t + 1))
                    osb = outp.tile([128, D], BF16, tag="osb")
                    for nh in range(2):
                        po = psA.tile([128, QW], F32, tag="pa")
                        nsl = slice(QW * nh, QW * (nh + 1))
                        nc.tensor.matmul(
                            po, ctxn[:, 0, tsl], wo[:, 0, nsl], start=True, stop=False
                        )
                        nc.tensor.matmul(
                            po, ctxn[:, 1, tsl], wo[:, 1, nsl], start=False, stop=True
                        )
                        if tail and nh == 1:
                            nc.scalar.copy(osb[:, nsl], po)
                        else:
                            nc.vector.tensor_copy(osb[:, nsl], po)
                    eng = (nc.scalar if t % 2 else nc.sync) if tail else nc.sync
                    eng.dma_start(out=out_d.ap()[tsl, :], in_=osb)
                return go

            # ---- attention pipeline over "units" (off-diag kp pairs / diag slots)
            pend = []

            def av(it):
                if it["kind"] == "pair":
                    for h, ctx_t in ((0, it["ctxa"]), (1, it["ctxb"])):
                        nc.tensor.matmul(
                            ctx_t,
                            vext8[:, it["m"], 2 * it["p"] + h, :, :],
                            it["pt8"][:, h, :, :],
                            start=it["first"],
                            stop=it["lastu"],
                            perf_mode=DR,
                        )
                elif it["full"]:
                    # qc==0: full-width diag AV (pt zero-padded) so start/stop
                    # accumulation flags always cover the whole [VW, QW] region
                    for h, ctx_t in ((0, it["ctxa"]), (1, it["ctxb"])):
                        nc.tensor.matmul(
                            ctx_t,
                            vextb[:, it["kp"], 2 * it["p"] + h, :],
                            it["pt"][:, h, :],
                            start=it["first"],
                            stop=it["lastu"],
                        )
                else:
                    j = it["j"]
                    w = QW - 128 * j
                    for h, ctx_t in ((0, it["ctxa"]), (1, it["ctxb"])):
                        nc.tensor.matmul(
                            ctx_t[:, 128 * j : QW],
                            vextb[:, it["kp"], 2 * it["p"] + h, :],
                            it["pt"][:, h, 0:w],
                            start=it["first"],
                            stop=it["lastu"],
                        )

            def norm(it):
                qc, p = it["qc"], it["p"]
                last = qc == 3 and p == 1
                qsl = slice(qc * QW, (qc + 1) * QW)
                for h, ctx_t in ((0, it["ctxa"]), (1, it["ctxb"])):
                    rec1 = recp.tile([1, QW], F32, tag="rec")
                    nc.vector.tensor_copy(rec1, ctx_t[HD : HD + 1, :])
                    recr = recp.tile([1, QW], F32, tag="recr")
                    nc.vector.reciprocal_approx_fast(recr, rec1)
                    rb = rbp.tile([HD, QW], F32, tag="rb")
                    nc.gpsimd.partition_broadcast(rb, recr)
                    nc.vector.scalar_tensor_tensor(
                        out=ctxn[64 * h : 64 * h + 64, p, qsl],
                        in0=ctx_t[0:HD, :],
                        scalar=1.0,
                        in1=rb,
                        op0=OP.mult,
                        op1=OP.mult,
                    )

            def flush_one():
                it = pend.pop(0)
                av(it)
                if it["lastu"]:
                    norm(it)
                    if it["p"] == 1:
                        tail = it["qc"] == 3
                        host = OUT_HOST[it["qc"]]
                        for t in range(4 * it["qc"], 4 * it["qc"] + 4):
                            fq[host].append(mk_outproj(t, tail=tail))

            # ---- prelude + filler plan
            mk_qtkt(qt, wqh, 0, 0)()
            mk_qtkt(kt, wkh, 0, 0)()
            fq[(0, 0)] = [mk_v(0), mk_v(1), mk_v(2), mk_v(3),
                          mk_qtkt(qt, wqh, 1, 0), mk_qtkt(kt, wkh, 1, 0)]
            fq[(0, 1)] = [mk_qtkt(qt, wqh, 0, 1), mk_qtkt(kt, wkh, 0, 1),
                          mk_v8cast(0), mk_v8cast(1), mk_v8cast(2), mk_v8cast(3)]
            fq[(1, 0)] = [mk_v(4), mk_v(5), mk_v(6), mk_v(7),
                          mk_qtkt(qt, wqh, 1, 1), mk_qtkt(kt, wkh, 1, 1)]
            fq[(1, 1)] = [mk_qtkt(qt, wqh, 0, 2), mk_qtkt(kt, wkh, 0, 2),
                          mk_v8cast(4), mk_v8cast(5), mk_v8cast(6), mk_v8cast(7)]
            fq[(2, 0)] = [mk_v(8), mk_v(9), mk_v(10), mk_v(11),
                          mk_qtkt(qt, wqh, 1, 2), mk_qtkt(kt, wkh, 1, 2)]
            fq[(2, 1)] = [mk_qtkt(qt, wqh, 0, 3), mk_qtkt(kt, wkh, 0, 3),
                          mk_v(12), mk_v(13),
                          mk_v8cast(8), mk_v8cast(9), mk_v8cast(10), mk_v8cast(11)]
            fq[(3, 0)] = [mk_v(14), mk_v(15),
                          mk_qtkt(qt, wqh, 1, 3), mk_qtkt(kt, wkh, 1, 3)]
            OUT_HOST = {0: (3, 1), 1: (3, 0), 2: (3, 1), 3: (3, 1)}

            def score_slot(qc, p, kp, pt8=None, par=0, ptd=None, full=False):
                """Score matmuls + exp for one kp slot."""
                qsl = slice(qc * QW, (qc + 1) * QW)
                ksl = slice(kp * 128, (kp + 1) * 128)
                diag = kp >= 4 * qc
                st_t = psS.tile([128, 2, QW], F32, tag="st")
                if not diag:
                    for h in (0, 1):
                        nc.tensor.matmul(
                            st_t[:, h, :],
                            kt[64 * h : 64 * h + 64, p, ksl],
                            qt[64 * h : 64 * h + 64, p, qsl],
                            start=True, stop=True,
                            tile_position=(64 * h, 0),
                        )
                    nc.scalar.activation(pt8[:, :, par, :], st_t, AF.Exp, scale=0.125)
                else:
                    j = kp - 4 * qc
                    w = QW - 128 * j
                    qtr = slice(qc * QW + 128 * j, (qc + 1) * QW)
                    for h in (0, 1):
                        nc.tensor.matmul(
                            st_t[:, h, 0:w],
                            kt[64 * h : 64 * h + 64, p, ksl],
                            qt[64 * h : 64 * h + 64, p, qtr],
                            start=True, stop=True,
                            tile_position=(64 * h, 0),
                        )
                    off = 128 * j if full else 0
                    if full and j > 0:
                        nc.gpsimd.memset(ptd[:, :, 0 : 128 * j], 0.0)
                    nc.scalar.activation(
                        ptd[:, :, off : off + w], st_t[:, :, 0:w], AF.Exp, scale=0.125
                    )
                    mw = min(w, 128)
                    nc.vector.tensor_mul(
                        ptd[:, :, off : off + mw],
                        ptd[:, :, off : off + mw],
                        maskb[:, :, 0:mw],
                    )

            # ---- main loop over passes
            for pi, (qc, p) in enumerate(PASS_ORDER):
                cur_pass[0] = pi
                drain_before(pi)
                full = qc == 0
                ctxa = psC.tile([VW, QW], F32, tag="ctx")
                ctxb = psC.tile([VW, QW], F32, tag="ctx")
                units = [("pair", m) for m in range(2 * qc)]
                units += [("diag", j) for j in range(4)]
                if not full and qc > 0:
                    # trimmed diag mode: a full-width pair must open and close
                    # the PSUM accumulation group
                    units.append(units.pop(2 * qc - 1))
                n_units = len(units)
                for ui, (kind, idx) in enumerate(units):
                    if pi == 0 and ui == 1:
                        x8dma(nc.sync, 1, 0, 4)
                        x8dma(nc.sync, 1, 4, 8)
                        xdma(nc.sync, 1, 0, 4)
                        xdma(nc.sync, 1, 4, 8)
                    if pi == 1 and ui == 0:
                        x8dma(nc.sync, 2, 0, 4)
                        x8dma(nc.sync, 2, 4, 8)
                        xdma(nc.sync, 2, 0, 4)
                        xdma(nc.sync, 2, 4, 8)
                    if pi == 2 and ui == 0:
                        x8dma(nc.sync, 3, 0, 4)
                        x8dma(nc.sync, 3, 4, 8)
                        xdma(nc.sync, 3, 0, 4)
                        xdma(nc.sync, 3, 4, 8)
                    common = dict(
                        qc=qc, p=p, ctxa=ctxa, ctxb=ctxb,
                        first=(ui == 0), lastu=(ui == n_units - 1),
                    )
                    if not (pi == 0 and ui == 0):
                        pop_filler(1)
                    while len(pend) > LAG:
                        flush_one()
                    if kind == "pair":
                        pt8 = pt8p.tile([128, 2, 2, QW], FP8, tag="pt8")
                        score_slot(qc, p, 2 * idx, pt8=pt8, par=0)
                        if pi < 4:
                            pop_filler(1)
                        score_slot(qc, p, 2 * idx + 1, pt8=pt8, par=1)
                        pend.append(dict(kind="pair", m=idx, pt8=pt8, **common))
                    else:
                        ptd = ptp.tile([128, 2, QW], BF16, tag="pt")
                        score_slot(qc, p, 4 * qc + idx, ptd=ptd, full=full)
                        pend.append(
                            dict(kind="diag", j=idx, kp=4 * qc + idx, pt=ptd,
                                 full=full, **common)
                        )
                    if pi < 4 and ui < 2:
                        pop_filler(1)
            while pend:
                pop_filler(1)
                flush_one()
            drain_before(len(PASS_ORDER))
            for pk in PASS_ORDER:
                for fn in fq[pk]:
                    fn()
                fq[pk].clear()

    nc.finalize()
    return nc


def _maskb():
    # multiplicative causal mask: 0 where q_local < kp_local, else 1
    m = np.ones((128, 128), dtype=np.float32)
    kp = np.arange(128)[:, None]
    q = np.arange(128)[None, :]
    m[q < kp] = 0.0
    return np.repeat(m[:, None, :], 2, axis=1).astype(ml_dtypes.bfloat16)


def shard_inputs(x, Wq, Wk, Wv, Wo):
    x = np.asarray(x, dtype=np.float32)
    Wq8 = np.asarray(Wq, dtype=np.float32).astype(ml_dtypes.float8_e4m3)
    Wk8 = np.asarray(Wk, dtype=np.float32).astype(ml_dtypes.float8_e4m3)
    Wv = np.asarray(Wv, dtype=ml_dtypes.bfloat16)
    Wo = np.asarray(Wo, dtype=ml_dtypes.bfloat16)
    mb = _maskb()
    xt = [np.ascontiguousarray(x[b].T) for b in range(2)]
    xt16 = [a.astype(ml_dtypes.bfloat16) for a in xt]
    xt8 = [a.astype(ml_dtypes.float8_e4m3) for a in xt]
    in_maps = []
    for c in range(8):
        b, hg = divmod(c, 4)
        sl = slice(DL * hg, DL * (hg + 1))
        in_maps.append({
            "x8": xt8[b],
            "xt": xt16[b],
            "wq": np.ascontiguousarray(Wq8[:, sl]),
            "wk": np.ascontiguousarray(Wk8[:, sl]),
            "wv": np.ascontiguousarray(Wv[:, sl]),
            "wo": np.ascontiguousarray(Wo[sl, :]),
            "maskb": mb,
        })
    return in_maps


def run(inputs, trace=False, **kwargs):
    """Build, run on 8 cores, and return (full_output, BassKernelResults)."""
    nc = build_bass()
    bo = np.asarray(inputs["bo"], dtype=np.float32)
    in_maps = shard_inputs(**{k: v for k, v in inputs.items() if k != "bo"})
    res = run_bass_kernel_spmd(
        nc, in_maps, core_ids=list(range(8)), trace=trace, **kwargs
    )
    parts = [np.asarray(r["out"], dtype=np.float32) for r in res.results]
    out = np.empty((2, S, D), dtype=np.float32)
    for b in range(2):
        out[b] = parts[4 * b] + parts[4 * b + 1] + parts[4 * b + 2] + parts[4 * b + 3]
        out[b] += bo[None, :]
    return out, res


def kernel(x, Wq, Wk, Wv, Wo, bo):
    out, _ = run(dict(x=x, Wq=Wq, Wk=Wk, Wv=Wv, Wo=Wo, bo=bo))
    return out


# revision 5
# speedup vs baseline: 1.0936x; 1.0936x over previous
"""Multi-head causal attention (B=2, S=2048, D=1024, H=16, d=64) on 8 trn2 cores.

Sharding: core c -> batch b=c//4, head-group hg=c%4 (4 heads, 256 of 1024 dims).
Each core computes its 4 heads' attention + its partial out-projection; host
sums the 4 partials per batch and adds the bias.

Design (from the bf16 baseline at ~156us to ~145us):
- q/k projections run as fp8e4m3 DoubleRow matmuls (x and Wq/Wk quantized to
  fp8 on the host, ic-chunk pairs packed per instruction) -> half the PE time
  for those matmuls. V projection and out-projection stay bf16 (rehearsed:
  V-path fp8 noise lands directly in the output).
- AV for OFF-DIAGONAL key tiles runs as fp8 DoubleRow over key-tile pairs:
  exp writes P straight to fp8, V is re-quantized to a 96-wide padded fp8
  copy (64 dims + ones column for the softmax denominator + zero pad;
  DoubleRow stationary width must be a multiple of 32). Diagonal tiles stay
  bf16: host rehearsal showed the fp8 error of the whole AV path lives in
  short rows that only touch diagonal blocks, so off-diagonal fp8 is free
  (adds 0.000 to rel err) while diagonal fp8 would blow the error budget.
- Diagonal V tiles are padded to 96 too so every AV matmul in a pass writes
  the same [96, 512] PSUM region (keeps start/stop accumulation groups
  legal); qc==0/1-style passes order units so a full-width matmul opens and
  closes the group.
- Causal mask multiply trimmed to the 128 columns containing the triangle.
- Scheduling: per-pass ordered filler queues move deferrable PE work
  (projections for later chunks, out-projection batches) into the ACT-bound
  late passes; fillers and due AV flushes are emitted BEFORE each unit's
  score matmuls so an exp-gated score never starves ready work behind it.
  Softmax denominator reciprocal runs on [1,512] before the partition
  broadcast; vext8 fp8 re-quant copies are separate fillers scheduled into
  DVE-idle windows.
- DMA: fp8 x/w startup tiles in ic-pair granularity on the sync queue (first
  DoubleRow matmul fires after ~160KB); wv/wo/bf16-x-chunk0 on the scalar
  queue before any exp is queued; later x chunks are issued 1-2 passes ahead
  of first use, all on the sync HWDGE queue (gpsimd-issued DMAs complete
  late; scalar-queue DMAs would block exp).
"""
import sys

sys.path.insert(0, "/opt/trn_rl_repo")

import numpy as np
import ml_dtypes
import concourse.bass as bass
import concourse.mybir as mybir
from concourse import bacc
from concourse.tile import TileContext
from concourse.bass_utils import run_bass_kernel_spmd

F32 = mybir.dt.float32
BF16 = mybir.dt.bfloat16
FP8 = mybir.dt.float8e4
AF = mybir.ActivationFunctionType
OP = mybir.AluOpType
DR = mybir.MatmulPerfMode.DoubleRow

S = 2048          # sequence length
D = 1024          # model dim
HD = 64           # head dim
NHL = 4           # heads per core
DL = 256          # local out dims (NHL * HD)
NQC = 4           # q chunks of 512
QW = 512          # q chunk width
NST = 16          # seq tiles of 128
NIC = 8           # input-dim chunks of 128
NPR = 4           # ic pairs (DoubleRow)
VW = 128          # vext width: 64 dims + 64 ones columns (denominator rows)
LAG = 3           # AV trails scores by this many units


def build_bass():
    nc = bacc.Bacc("TRN2", target_bir_lowering=False, debug=False, num_devices=8)

    x8_d = nc.dram_tensor("x8", [D, S], FP8, kind="ExternalInput")
    xt_d = nc.dram_tensor("xt", [D, S], BF16, kind="ExternalInput")
    wq_d = nc.dram_tensor("wq", [D, DL], FP8, kind="ExternalInput")
    wk_d = nc.dram_tensor("wk", [D, DL], FP8, kind="ExternalInput")
    wv_d = nc.dram_tensor("wv", [D, DL], BF16, kind="ExternalInput")
    wo_d = nc.dram_tensor("wo", [DL, D], BF16, kind="ExternalInput")
    mb_d = nc.dram_tensor("maskb", [128, 2, 128], BF16, kind="ExternalInput")
    out_d = nc.dram_tensor("out", [S, D], BF16, kind="ExternalOutput")

    with TileContext(nc) as tc:
        with (
            tc.tile_pool(name="consts", bufs=1) as consts,
            tc.tile_pool(name="xtp", bufs=1) as xtp,
            tc.tile_pool(name="qk", bufs=1) as qkp,
            tc.tile_pool(name="vv", bufs=1) as vvp,
            tc.tile_pool(name="ctxn", bufs=1) as ctxnp,
            tc.tile_pool(name="ptp", bufs=4) as ptp,
            tc.tile_pool(name="pt8p", bufs=4) as pt8p,
            tc.tile_pool(name="recp", bufs=4) as recp,
            tc.tile_pool(name="rbp", bufs=4) as rbp,
            tc.tile_pool(name="outp", bufs=3) as outp,
            tc.tile_pool(name="psA", bufs=2, space="PSUM") as psA,
            tc.tile_pool(name="psS", bufs=2, space="PSUM") as psS,
            tc.tile_pool(name="psC", bufs=2, space="PSUM") as psC,
        ):
            # ---- SBUF tiles
            # fp8 q/k weights: per p-half, per ic-pair tile [128, 2, 128]
            wqh = [
                [consts.tile([128, 2, 128], FP8, tag=f"wq{p}_{r}", name=f"wq{p}_{r}") for r in range(NPR)]
                for p in range(2)
            ]
            wkh = [
                [consts.tile([128, 2, 128], FP8, tag=f"wk{p}_{r}", name=f"wk{p}_{r}") for r in range(NPR)]
                for p in range(2)
            ]
            wv = consts.tile([128, NIC, DL], BF16, tag="wv")
            wo = consts.tile([128, 2, D], BF16, tag="wo")
            maskb = consts.tile([128, 2, 128], BF16, tag="maskb")
            onesr = consts.tile([1, HD], F32, tag="onesr")
            # fp8 x: chunk 0 in ic-pair tiles (startup), chunks 1-3 whole
            x8c0 = [xtp.tile([128, 2, QW], FP8, tag=f"x8c0_{r}", name=f"x8c0_{r}") for r in range(NPR)]
            x8s = [None] + [
                xtp.tile([128, NIC, QW], FP8, tag=f"x8_{c}", name=f"x8_{c}") for c in range(1, NQC)
            ]
            xts = [xtp.tile([128, NIC, QW], BF16, tag=f"xt{c}", name=f"xt{c}") for c in range(NQC)]
            qt = qkp.tile([128, 2, S], BF16, tag="qt")
            kt = qkp.tile([128, 2, S], BF16, tag="kt")
            # bf16 V (diag AV): [128, st, h, 96] (64 dims + ones + zero pad)
            vextb = vvp.tile([128, NST, NHL, VW], BF16, tag="vextb")
            # fp8 V (off-diag DR AV): key-tile pairs [128, m, h, par, 96]
            vext8 = vvp.tile([128, NST // 2, NHL, 2, VW], FP8, tag="vext8")
            ctxn = ctxnp.tile([128, 2, S], BF16, tag="ctxn")

            def x8pair(qc, r):
                return x8c0[r] if qc == 0 else x8s[qc][:, 2 * r : 2 * r + 2, :]

            # ---- DMAs.  Startup order: wq/x8 ic-pairs interleaved on sync so
            # the first DoubleRow matmul fires after ~160KB; wk pairs on scalar.
            def wpair_dma(eng, dst, src_d, p, r):
                eng.dma_start(
                    out=dst,
                    in_=src_d.ap()[256 * r : 256 * (r + 1), 128 * p : 128 * (p + 1)]
                    .rearrange("(c p) n -> p c n", p=128),
                )

            def x8c0_dma(eng, r):
                eng.dma_start(
                    out=x8c0[r],
                    in_=x8_d.ap()[256 * r : 256 * (r + 1), 0:QW].rearrange(
                        "(c p) s -> p c s", p=128
                    ),
                )

            def x8dma(eng, c, lo, hi):
                qsl = slice(c * QW, (c + 1) * QW)
                eng.dma_start(
                    out=x8s[c][:, lo:hi, :],
                    in_=x8_d.ap()[128 * lo : 128 * hi, qsl].rearrange(
                        "(c p) s -> p c s", p=128
                    ),
                )

            def xdma(eng, c, lo, hi):
                qsl = slice(c * QW, (c + 1) * QW)
                eng.dma_start(
                    out=xts[c][:, lo:hi, :],
                    in_=xt_d.ap()[128 * lo : 128 * hi, qsl].rearrange(
                        "(c p) s -> p c s", p=128
                    ),
                )

            for r in range(NPR):
                wpair_dma(nc.sync, wqh[0][r], wq_d, 0, r)
                x8c0_dma(nc.sync, r)
                wpair_dma(nc.scalar, wkh[0][r], wk_d, 0, r)
            xdma(nc.scalar, 0, 0, 4)
            nc.scalar.dma_start(out=wv, in_=wv_d.ap().rearrange("(c p) n -> p c n", p=128))
            xdma(nc.scalar, 0, 4, 8)
            for r in range(NPR):
                wpair_dma(nc.sync, wqh[1][r], wq_d, 1, r)
                wpair_dma(nc.sync, wkh[1][r], wk_d, 1, r)
            nc.scalar.dma_start(
                out=wo, in_=wo_d.ap().rearrange("(c p) n -> p c n", p=128)
            )
            nc.scalar.dma_start(out=maskb, in_=mb_d.ap())
            nc.vector.memset(onesr, 1.0)
            # 64 ones columns: the AV matmul replicates the softmax
            # denominator across PSUM rows 64..127, so the norm needs no
            # copy/broadcast - just a [64,512] reciprocal + multiply
            nc.vector.memset(vextb[:, :, :, HD:VW], 1.0)
            nc.gpsimd.memset(vext8[:, :, :, :, HD:VW], 1.0)

            # ---- PE filler scheduling: per-pass ordered queues + overflow.
            # Pass order interleaves the ACT-heavy qc=3 passes mid-kernel so
            # deferred projections/out-projections can feed the PE there.
            PASS_ORDER = [(0, 0), (0, 1), (1, 0), (1, 1), (2, 0), (2, 1), (3, 0), (3, 1)]
            fq = {pk: [] for pk in PASS_ORDER}
            overflow = []
            cur_pass = [0]

            def pop_filler(n=1):
                for _ in range(n):
                    q = fq[PASS_ORDER[cur_pass[0]]]
                    if q:
                        q.pop(0)()
                    elif overflow:
                        overflow.pop(0)()
                    else:
                        return

            def drain_before(pi):
                for pk in PASS_ORDER[:pi]:
                    for fn in fq[pk]:
                        fn()
                    fq[pk].clear()
                while overflow:
                    overflow.pop(0)()

            def mk_qtkt(dst, whs, p, qc):
                def go():
                    acc = psA.tile([128, QW], F32, tag="pa")
                    for r in range(NPR):
                        nc.tensor.matmul(
                            acc,
                            whs[p][r],
                            x8pair(qc, r),
                            start=(r == 0),
                            stop=(r == NPR - 1),
                            perf_mode=DR,
                        )
                    nc.vector.tensor_copy(dst[:, p, qc * QW : (qc + 1) * QW], acc)
                return go

            def mk_v(st):
                def go():
                    c, r = divmod(st, 4)
                    acc_t = psA.tile([128, QW], F32, tag="pa")
                    acc = acc_t[:, 0:DL]
                    for ic in range(NIC):
                        nc.tensor.matmul(
                            acc,
                            xts[c][:, ic, 128 * r : 128 * (r + 1)],
                            wv[:, ic, :],
                            start=(ic == 0),
                            stop=(ic == NIC - 1),
                        )
                    nc.vector.tensor_copy(
                        vextb[:, st, :, 0:HD], acc.rearrange("p (h e) -> p h e", h=NHL)
                    )
                return go

            def mk_v8cast(st):
                def go():
                    nc.vector.tensor_copy(
                        vext8[:, st // 2, :, st % 2, 0:HD], vextb[:, st, :, 0:HD]
                    )
                return go

            def mk_outproj(t, tail=False):
                def go():
                    tsl = slice(128 * t, 128 * (t + 1))
                    osb = outp.tile([128, D], BF16, tag="osb")
                    for nh in range(2):
                        po = psA.tile([128, QW], F32, tag="pa")
                        nsl = slice(QW * nh, QW * (nh + 1))
                        nc.tensor.matmul(
                            po, ctxn[:, 0, tsl], wo[:, 0, nsl], start=True, stop=False
                        )
                        nc.tensor.matmul(
                            po, ctxn[:, 1, tsl], wo[:, 1, nsl], start=False, stop=True
                        )
                        if tail and nh == 1:
                            nc.scalar.copy(osb[:, nsl], po)
                        else:
                            nc.vector.tensor_copy(osb[:, nsl], po)
                    eng = (nc.scalar if t % 2 else nc.sync) if tail else nc.sync
                    eng.dma_start(out=out_d.ap()[tsl, :], in_=osb)
                return go

            # ---- attention pipeline over "units" (off-diag kp pairs / diag slots)
            pend = []

            def av(it):
                if it["kind"] == "pair":
                    for h, ctx_t in ((0, it["ctxa"]), (1, it["ctxb"])):
                        nc.tensor.matmul(
                            ctx_t,
                            vext8[:, it["m"], 2 * it["p"] + h, :, :],
                            it["pt8"][:, h, :, :],
                            start=it["first"],
                            stop=it["lastu"],
                            perf_mode=DR,
                        )
                elif it["full"]:
                    # qc==0: full-width diag AV (pt zero-padded) so start/stop
                    # accumulation flags always cover the whole [VW, QW] region
                    for h, ctx_t in ((0, it["ctxa"]), (1, it["ctxb"])):
                        nc.tensor.matmul(
                            ctx_t,
                            vextb[:, it["kp"], 2 * it["p"] + h, :],
                            it["pt"][:, h, :],
                            start=it["first"],
                            stop=it["lastu"],
                        )
                else:
                    j = it["j"]
                    w = QW - 128 * j
                    for h, ctx_t in ((0, it["ctxa"]), (1, it["ctxb"])):
                        nc.tensor.matmul(
                            ctx_t[:, 128 * j : QW],
                            vextb[:, it["kp"], 2 * it["p"] + h, :],
                            it["pt"][:, h, 0:w],
                            start=it["first"],
                            stop=it["lastu"],
                        )

            def norm(it):
                qc, p = it["qc"], it["p"]
                last = qc == 3 and p == 1
                qsl = slice(qc * QW, (qc + 1) * QW)
                for h, ctx_t in ((0, it["ctxa"]), (1, it["ctxb"])):
                    rb0 = rbp.tile([HD, QW], F32, tag="rb0")
                    nc.vector.tensor_copy(rb0, ctx_t[HD : 2 * HD, :])
                    rb = rbp.tile([HD, QW], F32, tag="rb")
                    nc.vector.reciprocal_approx_fast(rb, rb0)
                    nc.vector.scalar_tensor_tensor(
                        out=ctxn[64 * h : 64 * h + 64, p, qsl],
                        in0=ctx_t[0:HD, :],
                        scalar=1.0,
                        in1=rb,
                        op0=OP.mult,
                        op1=OP.mult,
                    )

            def flush_one():
                it = pend.pop(0)
                av(it)
                if it["lastu"]:
                    norm(it)
                    if it["p"] == 1:
                        tail = it["qc"] == 3
                        host = OUT_HOST[it["qc"]]
                        for t in range(4 * it["qc"], 4 * it["qc"] + 4):
                            fq[host].append(mk_outproj(t, tail=tail))

            # ---- prelude + filler plan
            mk_qtkt(qt, wqh, 0, 0)()
            mk_qtkt(kt, wkh, 0, 0)()
            fq[(0, 0)] = [mk_v(0), mk_v(1), mk_v(2), mk_v(3),
                          mk_qtkt(qt, wqh, 1, 0), mk_qtkt(kt, wkh, 1, 0)]
            fq[(0, 1)] = [mk_qtkt(qt, wqh, 0, 1), mk_qtkt(kt, wkh, 0, 1),
                          mk_v8cast(0), mk_v8cast(1), mk_v8cast(2), mk_v8cast(3)]
            fq[(1, 0)] = [mk_v(4), mk_v(5), mk_v(6), mk_v(7),
                          mk_qtkt(qt, wqh, 1, 1), mk_qtkt(kt, wkh, 1, 1)]
            fq[(1, 1)] = [mk_qtkt(qt, wqh, 0, 2), mk_qtkt(kt, wkh, 0, 2),
                          mk_v8cast(4), mk_v8cast(5), mk_v8cast(6), mk_v8cast(7)]
            fq[(2, 0)] = [mk_v(8), mk_v(9), mk_v(10), mk_v(11),
                          mk_qtkt(qt, wqh, 1, 2), mk_qtkt(kt, wkh, 1, 2)]
            fq[(2, 1)] = [mk_qtkt(qt, wqh, 0, 3), mk_qtkt(kt, wkh, 0, 3),
                          mk_v(12), mk_v(13),
                          mk_v8cast(8), mk_v8cast(9), mk_v8cast(10), mk_v8cast(11)]
            fq[(3, 0)] = [mk_v(14), mk_v(15),
                          mk_qtkt(qt, wqh, 1, 3), mk_qtkt(kt, wkh, 1, 3)]
            OUT_HOST = {0: (3, 1), 1: (3, 0), 2: (3, 1), 3: (3, 1)}

            def score_slot(qc, p, kp, pt8=None, par=0, ptd=None, full=False):
                """Score matmuls + exp for one kp slot."""
                qsl = slice(qc * QW, (qc + 1) * QW)
                ksl = slice(kp * 128, (kp + 1) * 128)
                diag = kp >= 4 * qc
                st_t = psS.tile([128, 2, QW], F32, tag="st")
                if not diag:
                    for h in (0, 1):
                        nc.tensor.matmul(
                            st_t[:, h, :],
                            kt[64 * h : 64 * h + 64, p, ksl],
                            qt[64 * h : 64 * h + 64, p, qsl],
                            start=True, stop=True,
                            tile_position=(64 * h, 0),
                        )
                    nc.scalar.activation(pt8[:, :, par, :], st_t, AF.Exp, scale=0.125)
                else:
                    j = kp - 4 * qc
                    w = QW - 128 * j
                    qtr = slice(qc * QW + 128 * j, (qc + 1) * QW)
                    for h in (0, 1):
                        nc.tensor.matmul(
                            st_t[:, h, 0:w],
                            kt[64 * h : 64 * h + 64, p, ksl],
                            qt[64 * h : 64 * h + 64, p, qtr],
                            start=True, stop=True,
                            tile_position=(64 * h, 0),
                        )
                    off = 128 * j if full else 0
                    if full and j > 0:
                        nc.gpsimd.memset(ptd[:, :, 0 : 128 * j], 0.0)
                    nc.scalar.activation(
                        ptd[:, :, off : off + w], st_t[:, :, 0:w], AF.Exp, scale=0.125
                    )
                    mw = min(w, 128)
                    nc.vector.tensor_mul(
                        ptd[:, :, off : off + mw],
                        ptd[:, :, off : off + mw],
                        maskb[:, :, 0:mw],
                    )

            # ---- main loop over passes
            for pi, (qc, p) in enumerate(PASS_ORDER):
                cur_pass[0] = pi
                drain_before(pi)
                full = qc == 0
                ctxa = psC.tile([VW, QW], F32, tag="ctx")
                ctxb = psC.tile([VW, QW], F32, tag="ctx")
                units = [("pair", m) for m in range(2 * qc)]
                units += [("diag", j) for j in range(4)]
                if not full and qc > 0:
                    # trimmed diag mode: a full-width pair must open and close
                    # the PSUM accumulation group
                    units.append(units.pop(2 * qc - 1))
                n_units = len(units)
                for ui, (kind, idx) in enumerate(units):
                    if pi == 0 and ui == 1:
                        x8dma(nc.sync, 1, 0, 4)
                        x8dma(nc.sync, 1, 4, 8)
                        xdma(nc.sync, 1, 0, 4)
                        xdma(nc.sync, 1, 4, 8)
                    if pi == 1 and ui == 0:
                        x8dma(nc.sync, 2, 0, 4)
                        x8dma(nc.sync, 2, 4, 8)
                        xdma(nc.sync, 2, 0, 4)
                        xdma(nc.sync, 2, 4, 8)
                    if pi == 2 and ui == 0:
                        x8dma(nc.sync, 3, 0, 4)
                        x8dma(nc.sync, 3, 4, 8)
                        xdma(nc.sync, 3, 0, 4)
                        xdma(nc.sync, 3, 4, 8)
                    common = dict(
                        qc=qc, p=p, ctxa=ctxa, ctxb=ctxb,
                        first=(ui == 0), lastu=(ui == n_units - 1),
                    )
                    while len(pend) > LAG:
                        flush_one()
                    if not (pi == 0 and ui == 0):
                        pop_filler(1)
                    if kind == "pair":
                        pt8 = pt8p.tile([128, 2, 2, QW], FP8, tag="pt8")
                        score_slot(qc, p, 2 * idx, pt8=pt8, par=0)
                        if pi < 4:
                            pop_filler(1)
                        score_slot(qc, p, 2 * idx + 1, pt8=pt8, par=1)
                        pend.append(dict(kind="pair", m=idx, pt8=pt8, **common))
                    else:
                        ptd = ptp.tile([128, 2, QW], BF16, tag="pt")
                        score_slot(qc, p, 4 * qc + idx, ptd=ptd, full=full)
                        pend.append(
                            dict(kind="diag", j=idx, kp=4 * qc + idx, pt=ptd,
                                 full=full, **common)
                        )
                    if pi < 4 and ui < 2:
                        pop_filler(1)
            while pend:
                pop_filler(1)
                flush_one()
            drain_before(len(PASS_ORDER))
            for pk in PASS_ORDER:
                for fn in fq[pk]:
                    fn()
                fq[pk].clear()

    nc.finalize()
    return nc


def _maskb():
    # multiplicative causal mask: 0 where q_local < kp_local, else 1
    m = np.ones((128, 128), dtype=np.float32)
    kp = np.arange(128)[:, None]
    q = np.arange(128)[None, :]
    m[q < kp] = 0.0
    return np.repeat(m[:, None, :], 2, axis=1).astype(ml_dtypes.bfloat16)


def shard_inputs(x, Wq, Wk, Wv, Wo):
    x = np.asarray(x, dtype=np.float32)
    Wq8 = np.asarray(Wq, dtype=np.float32).astype(ml_dtypes.float8_e4m3)
    Wk8 = np.asarray(Wk, dtype=np.float32).astype(ml_dtypes.float8_e4m3)
    Wv = np.asarray(Wv, dtype=ml_dtypes.bfloat16)
    Wo = np.asarray(Wo, dtype=ml_dtypes.bfloat16)
    mb = _maskb()
    xt = [np.ascontiguousarray(x[b].T) for b in range(2)]
    xt16 = [a.astype(ml_dtypes.bfloat16) for a in xt]
    xt8 = [a.astype(ml_dtypes.float8_e4m3) for a in xt]
    in_maps = []
    for c in range(8):
        b, hg = divmod(c, 4)
        sl = slice(DL * hg, DL * (hg + 1))
        in_maps.append({
            "x8": xt8[b],
            "xt": xt16[b],
            "wq": np.ascontiguousarray(Wq8[:, sl]),
            "wk": np.ascontiguousarray(Wk8[:, sl]),
            "wv": np.ascontiguousarray(Wv[:, sl]),
            "wo": np.ascontiguousarray(Wo[sl, :]),
            "maskb": mb,
        })
    return in_maps


def run(inputs, trace=False, **kwargs):
    """Build, run on 8 cores, and return (full_output, BassKernelResults)."""
    nc = build_bass()
    bo = np.asarray(inputs["bo"], dtype=np.float32)
    in_maps = shard_inputs(**{k: v for k, v in inputs.items() if k != "bo"})
    res = run_bass_kernel_spmd(
        nc, in_maps, core_ids=list(range(8)), trace=trace, **kwargs
    )
    parts = [np.asarray(r["out"], dtype=np.float32) for r in res.results]
    out = np.empty((2, S, D), dtype=np.float32)
    for b in range(2):
        out[b] = parts[4 * b] + parts[4 * b + 1] + parts[4 * b + 2] + parts[4 * b + 3]
        out[b] += bo[None, :]
    return out, res


def kernel(x, Wq, Wk, Wv, Wo, bo):
    out, _ = run(dict(x=x, Wq=Wq, Wk=Wk, Wv=Wv, Wo=Wo, bo=bo))
    return out


# revision 6
# speedup vs baseline: 1.0941x; 1.0005x over previous
"""Multi-head causal attention (B=2, S=2048, D=1024, H=16, d=64) on 8 trn2 cores.

Sharding: core c -> batch b=c//4, head-group hg=c%4 (4 heads, 256 of 1024 dims).
Each core computes its 4 heads' attention + its partial out-projection; host
sums the 4 partials per batch and adds the bias.

Design (from the bf16 baseline at ~156us to ~145us):
- q/k projections run as fp8e4m3 DoubleRow matmuls (x and Wq/Wk quantized to
  fp8 on the host, ic-chunk pairs packed per instruction) -> half the PE time
  for those matmuls. V projection and out-projection stay bf16 (rehearsed:
  V-path fp8 noise lands directly in the output).
- AV for OFF-DIAGONAL key tiles runs as fp8 DoubleRow over key-tile pairs:
  exp writes P straight to fp8, V is re-quantized to a 96-wide padded fp8
  copy (64 dims + ones column for the softmax denominator + zero pad;
  DoubleRow stationary width must be a multiple of 32). Diagonal tiles stay
  bf16: host rehearsal showed the fp8 error of the whole AV path lives in
  short rows that only touch diagonal blocks, so off-diagonal fp8 is free
  (adds 0.000 to rel err) while diagonal fp8 would blow the error budget.
- Diagonal V tiles are padded to 96 too so every AV matmul in a pass writes
  the same [96, 512] PSUM region (keeps start/stop accumulation groups
  legal); qc==0/1-style passes order units so a full-width matmul opens and
  closes the group.
- Causal mask multiply trimmed to the 128 columns containing the triangle.
- Scheduling: per-pass ordered filler queues move deferrable PE work
  (projections for later chunks, out-projection batches) into the ACT-bound
  late passes; fillers and due AV flushes are emitted BEFORE each unit's
  score matmuls so an exp-gated score never starves ready work behind it.
  Softmax denominator reciprocal runs on [1,512] before the partition
  broadcast; vext8 fp8 re-quant copies are separate fillers scheduled into
  DVE-idle windows.
- DMA: fp8 x/w startup tiles in ic-pair granularity on the sync queue (first
  DoubleRow matmul fires after ~160KB); wv/wo/bf16-x-chunk0 on the scalar
  queue before any exp is queued; later x chunks are issued 1-2 passes ahead
  of first use, all on the sync HWDGE queue (gpsimd-issued DMAs complete
  late; scalar-queue DMAs would block exp).
"""
import sys

sys.path.insert(0, "/opt/trn_rl_repo")

import numpy as np
import ml_dtypes
import concourse.bass as bass
import concourse.mybir as mybir
from concourse import bacc
from concourse.tile import TileContext
from concourse.bass_utils import run_bass_kernel_spmd

F32 = mybir.dt.float32
BF16 = mybir.dt.bfloat16
FP8 = mybir.dt.float8e4
AF = mybir.ActivationFunctionType
OP = mybir.AluOpType
DR = mybir.MatmulPerfMode.DoubleRow

S = 2048          # sequence length
D = 1024          # model dim
HD = 64           # head dim
NHL = 4           # heads per core
DL = 256          # local out dims (NHL * HD)
NQC = 4           # q chunks of 512
QW = 512          # q chunk width
NST = 16          # seq tiles of 128
NIC = 8           # input-dim chunks of 128
NPR = 4           # ic pairs (DoubleRow)
VW = 128          # vext width: 64 dims + 64 ones columns (denominator rows)
LAG = 3           # AV trails scores by this many units


def build_bass():
    nc = bacc.Bacc("TRN2", target_bir_lowering=False, debug=False, num_devices=8)

    x8_d = nc.dram_tensor("x8", [D, S], FP8, kind="ExternalInput")
    xt_d = nc.dram_tensor("xt", [D, S], BF16, kind="ExternalInput")
    wq_d = nc.dram_tensor("wq", [D, DL], FP8, kind="ExternalInput")
    wk_d = nc.dram_tensor("wk", [D, DL], FP8, kind="ExternalInput")
    wv_d = nc.dram_tensor("wv", [D, DL], BF16, kind="ExternalInput")
    wo_d = nc.dram_tensor("wo", [DL, D], BF16, kind="ExternalInput")
    mb_d = nc.dram_tensor("maskb", [128, 2, 128], BF16, kind="ExternalInput")
    out_d = nc.dram_tensor("out", [S, D], BF16, kind="ExternalOutput")

    with TileContext(nc) as tc:
        with (
            tc.tile_pool(name="consts", bufs=1) as consts,
            tc.tile_pool(name="xtp", bufs=1) as xtp,
            tc.tile_pool(name="qk", bufs=1) as qkp,
            tc.tile_pool(name="vv", bufs=1) as vvp,
            tc.tile_pool(name="ctxn", bufs=1) as ctxnp,
            tc.tile_pool(name="ptp", bufs=4) as ptp,
            tc.tile_pool(name="pt8p", bufs=4) as pt8p,
            tc.tile_pool(name="recp", bufs=4) as recp,
            tc.tile_pool(name="rbp", bufs=4) as rbp,
            tc.tile_pool(name="outp", bufs=3) as outp,
            tc.tile_pool(name="psA", bufs=2, space="PSUM") as psA,
            tc.tile_pool(name="psS", bufs=2, space="PSUM") as psS,
            tc.tile_pool(name="psC", bufs=2, space="PSUM") as psC,
        ):
            # ---- SBUF tiles
            # fp8 q/k weights: per p-half, per ic-pair tile [128, 2, 128]
            wqh = [
                [consts.tile([128, 2, 128], FP8, tag=f"wq{p}_{r}", name=f"wq{p}_{r}") for r in range(NPR)]
                for p in range(2)
            ]
            wkh = [
                [consts.tile([128, 2, 128], FP8, tag=f"wk{p}_{r}", name=f"wk{p}_{r}") for r in range(NPR)]
                for p in range(2)
            ]
            wv = consts.tile([128, NIC, DL], BF16, tag="wv")
            wo = consts.tile([128, 2, D], BF16, tag="wo")
            maskb = consts.tile([128, 2, 128], BF16, tag="maskb")
            onesr = consts.tile([1, HD], F32, tag="onesr")
            # fp8 x: chunk 0 in ic-pair tiles (startup), chunks 1-3 whole
            x8c0 = [xtp.tile([128, 2, QW], FP8, tag=f"x8c0_{r}", name=f"x8c0_{r}") for r in range(NPR)]
            x8s = [None] + [
                xtp.tile([128, NIC, QW], FP8, tag=f"x8_{c}", name=f"x8_{c}") for c in range(1, NQC)
            ]
            xts = [xtp.tile([128, NIC, QW], BF16, tag=f"xt{c}", name=f"xt{c}") for c in range(NQC)]
            qt = qkp.tile([128, 2, S], BF16, tag="qt")
            kt = qkp.tile([128, 2, S], BF16, tag="kt")
            # bf16 V (diag AV): [128, st, h, 96] (64 dims + ones + zero pad)
            vextb = vvp.tile([128, NST, NHL, VW], BF16, tag="vextb")
            # fp8 V (off-diag DR AV): key-tile pairs [128, m, h, par, 96]
            vext8 = vvp.tile([128, NST // 2, NHL, 2, VW], FP8, tag="vext8")
            ctxn = ctxnp.tile([128, 2, S], BF16, tag="ctxn")

            def x8pair(qc, r):
                return x8c0[r] if qc == 0 else x8s[qc][:, 2 * r : 2 * r + 2, :]

            # ---- DMAs.  Startup order: wq/x8 ic-pairs interleaved on sync so
            # the first DoubleRow matmul fires after ~160KB; wk pairs on scalar.
            def wpair_dma(eng, dst, src_d, p, r):
                eng.dma_start(
                    out=dst,
                    in_=src_d.ap()[256 * r : 256 * (r + 1), 128 * p : 128 * (p + 1)]
                    .rearrange("(c p) n -> p c n", p=128),
                )

            def x8c0_dma(eng, r):
                eng.dma_start(
                    out=x8c0[r],
                    in_=x8_d.ap()[256 * r : 256 * (r + 1), 0:QW].rearrange(
                        "(c p) s -> p c s", p=128
                    ),
                )

            def x8dma(eng, c, lo, hi):
                qsl = slice(c * QW, (c + 1) * QW)
                eng.dma_start(
                    out=x8s[c][:, lo:hi, :],
                    in_=x8_d.ap()[128 * lo : 128 * hi, qsl].rearrange(
                        "(c p) s -> p c s", p=128
                    ),
                )

            def xdma(eng, c, lo, hi):
                qsl = slice(c * QW, (c + 1) * QW)
                eng.dma_start(
                    out=xts[c][:, lo:hi, :],
                    in_=xt_d.ap()[128 * lo : 128 * hi, qsl].rearrange(
                        "(c p) s -> p c s", p=128
                    ),
                )

            for r in range(NPR):
                wpair_dma(nc.sync, wqh[0][r], wq_d, 0, r)
                x8c0_dma(nc.sync, r)
                wpair_dma(nc.scalar, wkh[0][r], wk_d, 0, r)
            xdma(nc.scalar, 0, 0, 4)
            nc.scalar.dma_start(out=wv, in_=wv_d.ap().rearrange("(c p) n -> p c n", p=128))
            xdma(nc.scalar, 0, 4, 8)
            for r in range(NPR):
                wpair_dma(nc.sync, wqh[1][r], wq_d, 1, r)
                wpair_dma(nc.sync, wkh[1][r], wk_d, 1, r)
            nc.scalar.dma_start(
                out=wo, in_=wo_d.ap().rearrange("(c p) n -> p c n", p=128)
            )
            nc.scalar.dma_start(out=maskb, in_=mb_d.ap())
            nc.vector.memset(onesr, 1.0)
            # 64 ones columns: the AV matmul replicates the softmax
            # denominator across PSUM rows 64..127, so the norm needs no
            # copy/broadcast - just a [64,512] reciprocal + multiply
            nc.vector.memset(vextb[:, :, :, HD:VW], 1.0)
            nc.gpsimd.memset(vext8[:, :, :, :, HD:VW], 1.0)

            # ---- PE filler scheduling: per-pass ordered queues + overflow.
            # Pass order interleaves the ACT-heavy qc=3 passes mid-kernel so
            # deferred projections/out-projections can feed the PE there.
            PASS_ORDER = [(0, 0), (0, 1), (1, 0), (1, 1), (2, 0), (2, 1), (3, 0), (3, 1)]
            fq = {pk: [] for pk in PASS_ORDER}
            overflow = []
            cur_pass = [0]

            def pop_filler(n=1):
                for _ in range(n):
                    q = fq[PASS_ORDER[cur_pass[0]]]
                    if q:
                        q.pop(0)()
                    elif overflow:
                        overflow.pop(0)()
                    else:
                        return

            def drain_before(pi):
                for pk in PASS_ORDER[:pi]:
                    for fn in fq[pk]:
                        fn()
                    fq[pk].clear()
                while overflow:
                    overflow.pop(0)()

            def mk_qtkt(dst, whs, p, qc):
                def go():
                    acc = psA.tile([128, QW], F32, tag="pa")
                    for r in range(NPR):
                        nc.tensor.matmul(
                            acc,
                            whs[p][r],
                            x8pair(qc, r),
                            start=(r == 0),
                            stop=(r == NPR - 1),
                            perf_mode=DR,
                        )
                    nc.vector.tensor_copy(dst[:, p, qc * QW : (qc + 1) * QW], acc)
                return go

            def mk_v(st):
                def go():
                    c, r = divmod(st, 4)
                    acc_t = psA.tile([128, QW], F32, tag="pa")
                    acc = acc_t[:, 0:DL]
                    for ic in range(NIC):
                        nc.tensor.matmul(
                            acc,
                            xts[c][:, ic, 128 * r : 128 * (r + 1)],
                            wv[:, ic, :],
                            start=(ic == 0),
                            stop=(ic == NIC - 1),
                        )
                    nc.vector.tensor_copy(
                        vextb[:, st, :, 0:HD], acc.rearrange("p (h e) -> p h e", h=NHL)
                    )
                return go

            def mk_v8cast(st):
                def go():
                    nc.vector.tensor_copy(
                        vext8[:, st // 2, :, st % 2, 0:HD], vextb[:, st, :, 0:HD]
                    )
                return go

            def mk_outproj(t, tail=False):
                def go():
                    tsl = slice(128 * t, 128 * (t + 1))
                    osb = outp.tile([128, D], BF16, tag="osb")
                    for nh in range(2):
                        po = psA.tile([128, QW], F32, tag="pa")
                        nsl = slice(QW * nh, QW * (nh + 1))
                        nc.tensor.matmul(
                            po, ctxn[:, 0, tsl], wo[:, 0, nsl], start=True, stop=False
                        )
                        nc.tensor.matmul(
                            po, ctxn[:, 1, tsl], wo[:, 1, nsl], start=False, stop=True
                        )
                        if tail and nh == 1:
                            nc.scalar.copy(osb[:, nsl], po)
                        else:
                            nc.vector.tensor_copy(osb[:, nsl], po)
                        if tail:
                            eng = nc.scalar if (t + nh) % 2 else nc.sync
                            eng.dma_start(out=out_d.ap()[tsl, nsl], in_=osb[:, nsl])
                    if not tail:
                        nc.sync.dma_start(out=out_d.ap()[tsl, :], in_=osb)
                return go

            # ---- attention pipeline over "units" (off-diag kp pairs / diag slots)
            pend = []

            def av(it):
                if it["kind"] == "pair":
                    for h, ctx_t in ((0, it["ctxa"]), (1, it["ctxb"])):
                        nc.tensor.matmul(
                            ctx_t,
                            vext8[:, it["m"], 2 * it["p"] + h, :, :],
                            it["pt8"][:, h, :, :],
                            start=it["first"],
                            stop=it["lastu"],
                            perf_mode=DR,
                        )
                elif it["full"]:
                    # qc==0: full-width diag AV (pt zero-padded) so start/stop
                    # accumulation flags always cover the whole [VW, QW] region
                    for h, ctx_t in ((0, it["ctxa"]), (1, it["ctxb"])):
                        nc.tensor.matmul(
                            ctx_t,
                            vextb[:, it["kp"], 2 * it["p"] + h, :],
                            it["pt"][:, h, :],
                            start=it["first"],
                            stop=it["lastu"],
                        )
                else:
                    j = it["j"]
                    w = QW - 128 * j
                    for h, ctx_t in ((0, it["ctxa"]), (1, it["ctxb"])):
                        nc.tensor.matmul(
                            ctx_t[:, 128 * j : QW],
                            vextb[:, it["kp"], 2 * it["p"] + h, :],
                            it["pt"][:, h, 0:w],
                            start=it["first"],
                            stop=it["lastu"],
                        )

            def norm(it):
                qc, p = it["qc"], it["p"]
                last = qc == 3 and p == 1
                qsl = slice(qc * QW, (qc + 1) * QW)
                for h, ctx_t in ((0, it["ctxa"]), (1, it["ctxb"])):
                    rb0 = rbp.tile([HD, QW], F32, tag="rb0")
                    nc.vector.tensor_copy(rb0, ctx_t[HD : 2 * HD, :])
                    rb = rbp.tile([HD, QW], F32, tag="rb")
                    nc.vector.reciprocal_approx_fast(rb, rb0)
                    nc.vector.scalar_tensor_tensor(
                        out=ctxn[64 * h : 64 * h + 64, p, qsl],
                        in0=ctx_t[0:HD, :],
                        scalar=1.0,
                        in1=rb,
                        op0=OP.mult,
                        op1=OP.mult,
                    )

            def flush_one():
                it = pend.pop(0)
                av(it)
                if it["lastu"]:
                    norm(it)
                    if it["p"] == 1:
                        tail = it["qc"] == 3
                        host = OUT_HOST[it["qc"]]
                        for ti, t in enumerate(range(4 * it["qc"], 4 * it["qc"] + 4)):
                            h2 = (3, 1) if (it["qc"] == 1 and ti >= 2) else host
                            fq[h2].append(mk_outproj(t, tail=tail))

            # ---- prelude + filler plan
            mk_qtkt(qt, wqh, 0, 0)()
            mk_qtkt(kt, wkh, 0, 0)()
            fq[(0, 0)] = [mk_v(0), mk_v(1), mk_v(2), mk_v(3),
                          mk_qtkt(qt, wqh, 1, 0), mk_qtkt(kt, wkh, 1, 0)]
            fq[(0, 1)] = [mk_qtkt(qt, wqh, 0, 1), mk_qtkt(kt, wkh, 0, 1),
                          mk_v8cast(0), mk_v8cast(1), mk_v8cast(2), mk_v8cast(3)]
            fq[(1, 0)] = [mk_v(4), mk_v(5), mk_v(6), mk_v(7),
                          mk_qtkt(qt, wqh, 1, 1), mk_qtkt(kt, wkh, 1, 1)]
            fq[(1, 1)] = [mk_qtkt(qt, wqh, 0, 2), mk_qtkt(kt, wkh, 0, 2),
                          mk_v8cast(4), mk_v8cast(5), mk_v8cast(6), mk_v8cast(7)]
            fq[(2, 0)] = [mk_v(8), mk_v(9), mk_v(10), mk_v(11),
                          mk_qtkt(qt, wqh, 1, 2), mk_qtkt(kt, wkh, 1, 2)]
            fq[(2, 1)] = [mk_qtkt(qt, wqh, 0, 3), mk_qtkt(kt, wkh, 0, 3),
                          mk_v(12), mk_v(13),
                          mk_v8cast(8), mk_v8cast(9), mk_v8cast(10), mk_v8cast(11)]
            fq[(3, 0)] = [mk_v(14), mk_v(15),
                          mk_qtkt(qt, wqh, 1, 3), mk_qtkt(kt, wkh, 1, 3)]
            OUT_HOST = {0: (3, 1), 1: (3, 0), 2: (3, 1), 3: (3, 1)}

            def score_slot(qc, p, kp, pt8=None, par=0, ptd=None, full=False):
                """Score matmuls + exp for one kp slot."""
                qsl = slice(qc * QW, (qc + 1) * QW)
                ksl = slice(kp * 128, (kp + 1) * 128)
                diag = kp >= 4 * qc
                st_t = psS.tile([128, 2, QW], F32, tag="st")
                if not diag:
                    for h in (0, 1):
                        nc.tensor.matmul(
                            st_t[:, h, :],
                            kt[64 * h : 64 * h + 64, p, ksl],
                            qt[64 * h : 64 * h + 64, p, qsl],
                            start=True, stop=True,
                            tile_position=(64 * h, 0),
                        )
                    nc.scalar.activation(pt8[:, :, par, :], st_t, AF.Exp, scale=0.125)
                else:
                    j = kp - 4 * qc
                    w = QW - 128 * j
                    qtr = slice(qc * QW + 128 * j, (qc + 1) * QW)
                    for h in (0, 1):
                        nc.tensor.matmul(
                            st_t[:, h, 0:w],
                            kt[64 * h : 64 * h + 64, p, ksl],
                            qt[64 * h : 64 * h + 64, p, qtr],
                            start=True, stop=True,
                            tile_position=(64 * h, 0),
                        )
                    off = 128 * j if full else 0
                    if full and j > 0:
                        nc.gpsimd.memset(ptd[:, :, 0 : 128 * j], 0.0)
                    nc.scalar.activation(
                        ptd[:, :, off : off + w], st_t[:, :, 0:w], AF.Exp, scale=0.125
                    )
                    mw = min(w, 128)
                    nc.vector.tensor_mul(
                        ptd[:, :, off : off + mw],
                        ptd[:, :, off : off + mw],
                        maskb[:, :, 0:mw],
                    )

            # ---- main loop over passes
            for pi, (qc, p) in enumerate(PASS_ORDER):
                cur_pass[0] = pi
                drain_before(pi)
                full = qc == 0
                ctxa = psC.tile([VW, QW], F32, tag="ctx")
                ctxb = psC.tile([VW, QW], F32, tag="ctx")
                units = [("pair", m) for m in range(2 * qc)]
                units += [("diag", j) for j in range(4)]
                if not full and qc > 0:
                    # trimmed diag mode: a full-width pair must open and close
                    # the PSUM accumulation group
                    units.append(units.pop(2 * qc - 1))
                n_units = len(units)
                for ui, (kind, idx) in enumerate(units):
                    if pi == 0 and ui == 1:
                        x8dma(nc.sync, 1, 0, 4)
                        x8dma(nc.sync, 1, 4, 8)
                        xdma(nc.sync, 1, 0, 4)
                        xdma(nc.sync, 1, 4, 8)
                    if pi == 1 and ui == 0:
                        x8dma(nc.sync, 2, 0, 4)
                        x8dma(nc.sync, 2, 4, 8)
                        xdma(nc.sync, 2, 0, 4)
                        xdma(nc.sync, 2, 4, 8)
                    if pi == 2 and ui == 0:
                        x8dma(nc.sync, 3, 0, 4)
                        x8dma(nc.sync, 3, 4, 8)
                        xdma(nc.sync, 3, 0, 4)
                        xdma(nc.sync, 3, 4, 8)
                    common = dict(
                        qc=qc, p=p, ctxa=ctxa, ctxb=ctxb,
                        first=(ui == 0), lastu=(ui == n_units - 1),
                    )
                    while len(pend) > LAG:
                        flush_one()
                    if not (pi == 0 and ui == 0):
                        pop_filler(1)
                    if kind == "pair":
                        pt8 = pt8p.tile([128, 2, 2, QW], FP8, tag="pt8")
                        score_slot(qc, p, 2 * idx, pt8=pt8, par=0)
                        if pi < 4:
                            pop_filler(1)
                        score_slot(qc, p, 2 * idx + 1, pt8=pt8, par=1)
                        pend.append(dict(kind="pair", m=idx, pt8=pt8, **common))
                    else:
                        ptd = ptp.tile([128, 2, QW], BF16, tag="pt")
                        score_slot(qc, p, 4 * qc + idx, ptd=ptd, full=full)
                        pend.append(
                            dict(kind="diag", j=idx, kp=4 * qc + idx, pt=ptd,
                                 full=full, **common)
                        )
                    if pi < 4 and ui < 2:
                        pop_filler(1)
            while pend:
                pop_filler(1)
                flush_one()
            drain_before(len(PASS_ORDER))
            for pk in PASS_ORDER:
                for fn in fq[pk]:
                    fn()
                fq[pk].clear()

    nc.finalize()
    return nc


def _maskb():
    # multiplicative causal mask: 0 where q_local < kp_local, else 1
    m = np.ones((128, 128), dtype=np.float32)
    kp = np.arange(128)[:, None]
    q = np.arange(128)[None, :]
    m[q < kp] = 0.0
    return np.repeat(m[:, None, :], 2, axis=1).astype(ml_dtypes.bfloat16)


def shard_inputs(x, Wq, Wk, Wv, Wo):
    x = np.asarray(x, dtype=np.float32)
    Wq8 = np.asarray(Wq, dtype=np.float32).astype(ml_dtypes.float8_e4m3)
    Wk8 = np.asarray(Wk, dtype=np.float32).astype(ml_dtypes.float8_e4m3)
    Wv = np.asarray(Wv, dtype=ml_dtypes.bfloat16)
    Wo = np.asarray(Wo, dtype=ml_dtypes.bfloat16)
    mb = _maskb()
    xt = [np.ascontiguousarray(x[b].T) for b in range(2)]
    xt16 = [a.astype(ml_dtypes.bfloat16) for a in xt]
    xt8 = [a.astype(ml_dtypes.float8_e4m3) for a in xt]
    in_maps = []
    for c in range(8):
        b, hg = divmod(c, 4)
        sl = slice(DL * hg, DL * (hg + 1))
        in_maps.append({
            "x8": xt8[b],
            "xt": xt16[b],
            "wq": np.ascontiguousarray(Wq8[:, sl]),
            "wk": np.ascontiguousarray(Wk8[:, sl]),
            "wv": np.ascontiguousarray(Wv[:, sl]),
            "wo": np.ascontiguousarray(Wo[sl, :]),
            "maskb": mb,
        })
    return in_maps


def run(inputs, trace=False, **kwargs):
    """Build, run on 8 cores, and return (full_output, BassKernelResults)."""
    nc = build_bass()
    bo = np.asarray(inputs["bo"], dtype=np.float32)
    in_maps = shard_inputs(**{k: v for k, v in inputs.items() if k != "bo"})
    res = run_bass_kernel_spmd(
        nc, in_maps, core_ids=list(range(8)), trace=trace, **kwargs
    )
    parts = [np.asarray(r["out"], dtype=np.float32) for r in res.results]
    out = np.empty((2, S, D), dtype=np.float32)
    for b in range(2):
        out[b] = parts[4 * b] + parts[4 * b + 1] + parts[4 * b + 2] + parts[4 * b + 3]
        out[b] += bo[None, :]
    return out, res


def kernel(x, Wq, Wk, Wv, Wo, bo):
    out, _ = run(dict(x=x, Wq=Wq, Wk=Wk, Wv=Wv, Wo=Wo, bo=bo))
    return out


# revision 7
# speedup vs baseline: 1.0977x; 1.0033x over previous
"""Multi-head causal attention (B=2, S=2048, D=1024, H=16, d=64) on 8 trn2 cores.

Sharding: core c -> batch b=c//4, head-group hg=c%4 (4 heads, 256 of 1024 dims).
Each core computes its 4 heads' attention + its partial out-projection; host
sums the 4 partials per batch and adds the bias.

Design (from the bf16 baseline at ~156us to ~145us):
- q/k projections run as fp8e4m3 DoubleRow matmuls (x and Wq/Wk quantized to
  fp8 on the host, ic-chunk pairs packed per instruction) -> half the PE time
  for those matmuls. V projection and out-projection stay bf16 (rehearsed:
  V-path fp8 noise lands directly in the output).
- AV for OFF-DIAGONAL key tiles runs as fp8 DoubleRow over key-tile pairs:
  exp writes P straight to fp8, V is re-quantized to a 96-wide padded fp8
  copy (64 dims + ones column for the softmax denominator + zero pad;
  DoubleRow stationary width must be a multiple of 32). Diagonal tiles stay
  bf16: host rehearsal showed the fp8 error of the whole AV path lives in
  short rows that only touch diagonal blocks, so off-diagonal fp8 is free
  (adds 0.000 to rel err) while diagonal fp8 would blow the error budget.
- Diagonal V tiles are padded to 96 too so every AV matmul in a pass writes
  the same [96, 512] PSUM region (keeps start/stop accumulation groups
  legal); qc==0/1-style passes order units so a full-width matmul opens and
  closes the group.
- Causal mask multiply trimmed to the 128 columns containing the triangle.
- Scheduling: per-pass ordered filler queues move deferrable PE work
  (projections for later chunks, out-projection batches) into the ACT-bound
  late passes; fillers and due AV flushes are emitted BEFORE each unit's
  score matmuls so an exp-gated score never starves ready work behind it.
  Softmax denominator reciprocal runs on [1,512] before the partition
  broadcast; vext8 fp8 re-quant copies are separate fillers scheduled into
  DVE-idle windows.
- DMA: fp8 x/w startup tiles in ic-pair granularity on the sync queue (first
  DoubleRow matmul fires after ~160KB); wv/wo/bf16-x-chunk0 on the scalar
  queue before any exp is queued; later x chunks are issued 1-2 passes ahead
  of first use, all on the sync HWDGE queue (gpsimd-issued DMAs complete
  late; scalar-queue DMAs would block exp).
"""
import sys

sys.path.insert(0, "/opt/trn_rl_repo")

import numpy as np
import ml_dtypes
import concourse.bass as bass
import concourse.mybir as mybir
from concourse import bacc
from concourse.tile import TileContext
from concourse.bass_utils import run_bass_kernel_spmd

F32 = mybir.dt.float32
BF16 = mybir.dt.bfloat16
FP8 = mybir.dt.float8e4
AF = mybir.ActivationFunctionType
OP = mybir.AluOpType
DR = mybir.MatmulPerfMode.DoubleRow

S = 2048          # sequence length
D = 1024          # model dim
HD = 64           # head dim
NHL = 4           # heads per core
DL = 256          # local out dims (NHL * HD)
NQC = 4           # q chunks of 512
QW = 512          # q chunk width
NST = 16          # seq tiles of 128
NIC = 8           # input-dim chunks of 128
NPR = 4           # ic pairs (DoubleRow)
VW = 128          # vext width: 64 dims + 64 ones columns (denominator rows)
LAG = 3           # AV trails scores by this many units


def build_bass():
    nc = bacc.Bacc("TRN2", target_bir_lowering=False, debug=False, num_devices=8)

    x8_d = nc.dram_tensor("x8", [D, S], FP8, kind="ExternalInput")
    xt_d = nc.dram_tensor("xt", [D, S], BF16, kind="ExternalInput")
    wq_d = nc.dram_tensor("wq", [D, DL], FP8, kind="ExternalInput")
    wk_d = nc.dram_tensor("wk", [D, DL], FP8, kind="ExternalInput")
    wv_d = nc.dram_tensor("wv", [D, DL], BF16, kind="ExternalInput")
    wo_d = nc.dram_tensor("wo", [DL, D], BF16, kind="ExternalInput")
    mb_d = nc.dram_tensor("maskb", [128, 2, 128], BF16, kind="ExternalInput")
    out_d = nc.dram_tensor("out", [S, D], BF16, kind="ExternalOutput")

    with TileContext(nc) as tc:
        with (
            tc.tile_pool(name="consts", bufs=1) as consts,
            tc.tile_pool(name="xtp", bufs=1) as xtp,
            tc.tile_pool(name="qk", bufs=1) as qkp,
            tc.tile_pool(name="vv", bufs=1) as vvp,
            tc.tile_pool(name="ctxn", bufs=1) as ctxnp,
            tc.tile_pool(name="ptp", bufs=4) as ptp,
            tc.tile_pool(name="pt8p", bufs=4) as pt8p,
            tc.tile_pool(name="recp", bufs=4) as recp,
            tc.tile_pool(name="rbp", bufs=4) as rbp,
            tc.tile_pool(name="outp", bufs=3) as outp,
            tc.tile_pool(name="psA", bufs=2, space="PSUM") as psA,
            tc.tile_pool(name="psS", bufs=2, space="PSUM") as psS,
            tc.tile_pool(name="psC", bufs=2, space="PSUM") as psC,
        ):
            # ---- SBUF tiles
            # fp8 q/k weights: per p-half, per ic-pair tile [128, 2, 128]
            wqh = [
                [consts.tile([128, 2, 128], FP8, tag=f"wq{p}_{r}", name=f"wq{p}_{r}") for r in range(NPR)]
                for p in range(2)
            ]
            wkh = [
                [consts.tile([128, 2, 128], FP8, tag=f"wk{p}_{r}", name=f"wk{p}_{r}") for r in range(NPR)]
                for p in range(2)
            ]
            wv = consts.tile([128, NIC, DL], BF16, tag="wv")
            wo = consts.tile([128, 2, D], BF16, tag="wo")
            maskb = consts.tile([128, 2, 128], BF16, tag="maskb")
            onesr = consts.tile([1, HD], F32, tag="onesr")
            # fp8 x: chunk 0 in ic-pair tiles (startup), chunks 1-3 whole
            x8c0 = [xtp.tile([128, 2, QW], FP8, tag=f"x8c0_{r}", name=f"x8c0_{r}") for r in range(NPR)]
            x8s = [None] + [
                xtp.tile([128, NIC, QW], FP8, tag=f"x8_{c}", name=f"x8_{c}") for c in range(1, NQC)
            ]
            xts = [xtp.tile([128, NIC, QW], BF16, tag=f"xt{c}", name=f"xt{c}") for c in range(NQC)]
            qt = qkp.tile([128, 2, S], BF16, tag="qt")
            kt = qkp.tile([128, 2, S], BF16, tag="kt")
            # bf16 V (diag AV): [128, st, h, 96] (64 dims + ones + zero pad)
            vextb = vvp.tile([128, NST, NHL, VW], BF16, tag="vextb")
            # fp8 V (off-diag DR AV): key-tile pairs [128, m, h, par, 96]
            vext8 = vvp.tile([128, NST // 2, NHL, 2, VW], FP8, tag="vext8")
            ctxn = ctxnp.tile([128, 2, S], BF16, tag="ctxn")

            def x8pair(qc, r):
                return x8c0[r] if qc == 0 else x8s[qc][:, 2 * r : 2 * r + 2, :]

            # ---- DMAs.  Startup order: wq/x8 ic-pairs interleaved on sync so
            # the first DoubleRow matmul fires after ~160KB; wk pairs on scalar.
            def wpair_dma(eng, dst, src_d, p, r):
                eng.dma_start(
                    out=dst,
                    in_=src_d.ap()[256 * r : 256 * (r + 1), 128 * p : 128 * (p + 1)]
                    .rearrange("(c p) n -> p c n", p=128),
                )

            def x8c0_dma(eng, r):
                eng.dma_start(
                    out=x8c0[r],
                    in_=x8_d.ap()[256 * r : 256 * (r + 1), 0:QW].rearrange(
                        "(c p) s -> p c s", p=128
                    ),
                )

            def x8dma(eng, c, lo, hi):
                qsl = slice(c * QW, (c + 1) * QW)
                eng.dma_start(
                    out=x8s[c][:, lo:hi, :],
                    in_=x8_d.ap()[128 * lo : 128 * hi, qsl].rearrange(
                        "(c p) s -> p c s", p=128
                    ),
                )

            def xdma(eng, c, lo, hi):
                qsl = slice(c * QW, (c + 1) * QW)
                eng.dma_start(
                    out=xts[c][:, lo:hi, :],
                    in_=xt_d.ap()[128 * lo : 128 * hi, qsl].rearrange(
                        "(c p) s -> p c s", p=128
                    ),
                )

            for r in range(NPR):
                wpair_dma(nc.sync, wqh[0][r], wq_d, 0, r)
                x8c0_dma(nc.sync, r)
                wpair_dma(nc.scalar, wkh[0][r], wk_d, 0, r)
            xdma(nc.scalar, 0, 0, 4)
            nc.scalar.dma_start(out=wv, in_=wv_d.ap().rearrange("(c p) n -> p c n", p=128))
            xdma(nc.scalar, 0, 4, 8)
            for r in range(NPR):
                wpair_dma(nc.sync, wqh[1][r], wq_d, 1, r)
                wpair_dma(nc.sync, wkh[1][r], wk_d, 1, r)
            nc.scalar.dma_start(out=maskb, in_=mb_d.ap())
            nc.vector.memset(onesr, 1.0)
            # 64 ones columns: the AV matmul replicates the softmax
            # denominator across PSUM rows 64..127, so the norm needs no
            # copy/broadcast - just a [64,512] reciprocal + multiply
            nc.vector.memset(vextb[:, :, :, HD:VW], 1.0)
            nc.gpsimd.memset(vext8[:, :, :, :, HD:VW], 1.0)

            # ---- PE filler scheduling: per-pass ordered queues + overflow.
            # Pass order interleaves the ACT-heavy qc=3 passes mid-kernel so
            # deferred projections/out-projections can feed the PE there.
            PASS_ORDER = [(0, 0), (0, 1), (1, 0), (1, 1), (2, 0), (2, 1), (3, 0), (3, 1)]
            fq = {pk: [] for pk in PASS_ORDER}
            overflow = []
            cur_pass = [0]

            def pop_filler(n=1):
                for _ in range(n):
                    q = fq[PASS_ORDER[cur_pass[0]]]
                    if q:
                        q.pop(0)()
                    elif overflow:
                        overflow.pop(0)()
                    else:
                        return

            def drain_before(pi):
                for pk in PASS_ORDER[:pi]:
                    for fn in fq[pk]:
                        fn()
                    fq[pk].clear()
                while overflow:
                    overflow.pop(0)()

            def mk_qtkt(dst, whs, p, qc):
                def go():
                    acc = psA.tile([128, QW], F32, tag="pa")
                    for r in range(NPR):
                        nc.tensor.matmul(
                            acc,
                            whs[p][r],
                            x8pair(qc, r),
                            start=(r == 0),
                            stop=(r == NPR - 1),
                            perf_mode=DR,
                        )
                    nc.vector.tensor_copy(dst[:, p, qc * QW : (qc + 1) * QW], acc)
                return go

            def mk_v(st):
                def go():
                    c, r = divmod(st, 4)
                    acc_t = psA.tile([128, QW], F32, tag="pa")
                    acc = acc_t[:, 0:DL]
                    for ic in range(NIC):
                        nc.tensor.matmul(
                            acc,
                            xts[c][:, ic, 128 * r : 128 * (r + 1)],
                            wv[:, ic, :],
                            start=(ic == 0),
                            stop=(ic == NIC - 1),
                        )
                    nc.vector.tensor_copy(
                        vextb[:, st, :, 0:HD], acc.rearrange("p (h e) -> p h e", h=NHL)
                    )
                return go

            def mk_v8cast(st):
                def go():
                    nc.vector.tensor_copy(
                        vext8[:, st // 2, :, st % 2, 0:HD], vextb[:, st, :, 0:HD]
                    )
                return go

            def mk_outproj(t, tail=False):
                def go():
                    tsl = slice(128 * t, 128 * (t + 1))
                    osb = outp.tile([128, D], BF16, tag="osb")
                    for nh in range(2):
                        po = psA.tile([128, QW], F32, tag="pa")
                        nsl = slice(QW * nh, QW * (nh + 1))
                        nc.tensor.matmul(
                            po, ctxn[:, 0, tsl], wo[:, 0, nsl], start=True, stop=False
                        )
                        nc.tensor.matmul(
                            po, ctxn[:, 1, tsl], wo[:, 1, nsl], start=False, stop=True
                        )
                        if tail and nh == 1:
                            nc.scalar.copy(osb[:, nsl], po)
                        else:
                            nc.vector.tensor_copy(osb[:, nsl], po)
                        if tail:
                            eng = nc.scalar if (t + nh) % 2 else nc.sync
                            eng.dma_start(out=out_d.ap()[tsl, nsl], in_=osb[:, nsl])
                    if not tail:
                        nc.sync.dma_start(out=out_d.ap()[tsl, :], in_=osb)
                return go

            # ---- attention pipeline over "units" (off-diag kp pairs / diag slots)
            pend = []

            def av(it):
                if it["kind"] == "pair":
                    for h, ctx_t in ((0, it["ctxa"]), (1, it["ctxb"])):
                        nc.tensor.matmul(
                            ctx_t,
                            vext8[:, it["m"], 2 * it["p"] + h, :, :],
                            it["pt8"][:, h, :, :],
                            start=it["first"],
                            stop=it["lastu"],
                            perf_mode=DR,
                        )
                elif it["full"]:
                    # qc==0: full-width diag AV (pt zero-padded) so start/stop
                    # accumulation flags always cover the whole [VW, QW] region
                    for h, ctx_t in ((0, it["ctxa"]), (1, it["ctxb"])):
                        nc.tensor.matmul(
                            ctx_t,
                            vextb[:, it["kp"], 2 * it["p"] + h, :],
                            it["pt"][:, h, :],
                            start=it["first"],
                            stop=it["lastu"],
                        )
                else:
                    j = it["j"]
                    w = QW - 128 * j
                    for h, ctx_t in ((0, it["ctxa"]), (1, it["ctxb"])):
                        nc.tensor.matmul(
                            ctx_t[:, 128 * j : QW],
                            vextb[:, it["kp"], 2 * it["p"] + h, :],
                            it["pt"][:, h, 0:w],
                            start=it["first"],
                            stop=it["lastu"],
                        )

            def norm(it):
                qc, p = it["qc"], it["p"]
                last = qc == 3 and p == 1
                qsl = slice(qc * QW, (qc + 1) * QW)
                for h, ctx_t in ((0, it["ctxa"]), (1, it["ctxb"])):
                    rb0 = rbp.tile([HD, QW], F32, tag="rb0")
                    nc.vector.tensor_copy(rb0, ctx_t[HD : 2 * HD, :])
                    rb = rbp.tile([HD, QW], F32, tag="rb")
                    nc.vector.reciprocal_approx_fast(rb, rb0)
                    nc.vector.scalar_tensor_tensor(
                        out=ctxn[64 * h : 64 * h + 64, p, qsl],
                        in0=ctx_t[0:HD, :],
                        scalar=1.0,
                        in1=rb,
                        op0=OP.mult,
                        op1=OP.mult,
                    )

            def flush_one():
                it = pend.pop(0)
                av(it)
                if it["lastu"]:
                    norm(it)
                    if it["p"] == 1:
                        tail = it["qc"] == 3
                        host = OUT_HOST[it["qc"]]
                        for ti, t in enumerate(range(4 * it["qc"], 4 * it["qc"] + 4)):
                            h2 = (3, 1) if (it["qc"] == 1 and ti >= 2) else host
                            fq[h2].append(mk_outproj(t, tail=tail))

            # ---- prelude + filler plan
            mk_qtkt(qt, wqh, 0, 0)()
            mk_qtkt(kt, wkh, 0, 0)()
            fq[(0, 0)] = [mk_v(0), mk_v(1), mk_v(2), mk_v(3),
                          mk_qtkt(qt, wqh, 1, 0), mk_qtkt(kt, wkh, 1, 0)]
            fq[(0, 1)] = [mk_qtkt(qt, wqh, 0, 1), mk_qtkt(kt, wkh, 0, 1),
                          mk_v8cast(0), mk_v8cast(1), mk_v8cast(2), mk_v8cast(3)]
            fq[(1, 0)] = [mk_v(4), mk_v(5), mk_v(6), mk_v(7),
                          mk_qtkt(qt, wqh, 1, 1), mk_qtkt(kt, wkh, 1, 1)]
            fq[(1, 1)] = [mk_qtkt(qt, wqh, 0, 2), mk_qtkt(kt, wkh, 0, 2),
                          mk_v8cast(4), mk_v8cast(5), mk_v8cast(6), mk_v8cast(7)]
            fq[(2, 0)] = [mk_v(8), mk_v(9), mk_v(10), mk_v(11),
                          mk_qtkt(qt, wqh, 1, 2), mk_qtkt(kt, wkh, 1, 2)]
            fq[(2, 1)] = [mk_qtkt(qt, wqh, 0, 3), mk_qtkt(kt, wkh, 0, 3),
                          mk_v(12), mk_v(13),
                          mk_v8cast(8), mk_v8cast(9), mk_v8cast(10), mk_v8cast(11)]
            fq[(3, 0)] = [mk_v(14), mk_v(15),
                          mk_qtkt(qt, wqh, 1, 3), mk_qtkt(kt, wkh, 1, 3)]
            OUT_HOST = {0: (3, 1), 1: (3, 0), 2: (3, 1), 3: (3, 1)}

            def score_slot(qc, p, kp, pt8=None, par=0, ptd=None, full=False):
                """Score matmuls + exp for one kp slot."""
                qsl = slice(qc * QW, (qc + 1) * QW)
                ksl = slice(kp * 128, (kp + 1) * 128)
                diag = kp >= 4 * qc
                st_t = psS.tile([128, 2, QW], F32, tag="st")
                if not diag:
                    for h in (0, 1):
                        nc.tensor.matmul(
                            st_t[:, h, :],
                            kt[64 * h : 64 * h + 64, p, ksl],
                            qt[64 * h : 64 * h + 64, p, qsl],
                            start=True, stop=True,
                            tile_position=(64 * h, 0),
                        )
                    nc.scalar.activation(pt8[:, :, par, :], st_t, AF.Exp, scale=0.125)
                else:
                    j = kp - 4 * qc
                    w = QW - 128 * j
                    qtr = slice(qc * QW + 128 * j, (qc + 1) * QW)
                    for h in (0, 1):
                        nc.tensor.matmul(
                            st_t[:, h, 0:w],
                            kt[64 * h : 64 * h + 64, p, ksl],
                            qt[64 * h : 64 * h + 64, p, qtr],
                            start=True, stop=True,
                            tile_position=(64 * h, 0),
                        )
                    off = 128 * j if full else 0
                    if full and j > 0:
                        nc.gpsimd.memset(ptd[:, :, 0 : 128 * j], 0.0)
                    nc.scalar.activation(
                        ptd[:, :, off : off + w], st_t[:, :, 0:w], AF.Exp, scale=0.125
                    )
                    mw = min(w, 128)
                    nc.vector.tensor_mul(
                        ptd[:, :, off : off + mw],
                        ptd[:, :, off : off + mw],
                        maskb[:, :, 0:mw],
                    )

            # ---- main loop over passes
            for pi, (qc, p) in enumerate(PASS_ORDER):
                cur_pass[0] = pi
                drain_before(pi)
                full = qc == 0
                ctxa = psC.tile([VW, QW], F32, tag="ctx")
                ctxb = psC.tile([VW, QW], F32, tag="ctx")
                units = [("pair", m) for m in range(2 * qc)]
                units += [("diag", j) for j in range(4)]
                if not full and qc > 0:
                    # trimmed diag mode: a full-width pair must open and close
                    # the PSUM accumulation group
                    units.append(units.pop(2 * qc - 1))
                n_units = len(units)
                for ui, (kind, idx) in enumerate(units):
                    if pi == 0 and ui == 1:
                        x8dma(nc.sync, 1, 0, 4)
                        x8dma(nc.sync, 1, 4, 8)
                        xdma(nc.sync, 1, 0, 4)
                        xdma(nc.sync, 1, 4, 8)
                        nc.sync.dma_start(
                            out=wo,
                            in_=wo_d.ap().rearrange("(c p) n -> p c n", p=128),
                        )
                    if pi == 1 and ui == 0:
                        x8dma(nc.sync, 2, 0, 4)
                        x8dma(nc.sync, 2, 4, 8)
                        xdma(nc.sync, 2, 0, 4)
                        xdma(nc.sync, 2, 4, 8)
                    if pi == 2 and ui == 0:
                        x8dma(nc.sync, 3, 0, 4)
                        x8dma(nc.sync, 3, 4, 8)
                        xdma(nc.sync, 3, 0, 4)
                        xdma(nc.sync, 3, 4, 8)
                    common = dict(
                        qc=qc, p=p, ctxa=ctxa, ctxb=ctxb,
                        first=(ui == 0), lastu=(ui == n_units - 1),
                    )
                    while len(pend) > LAG:
                        flush_one()
                    if not (pi == 0 and ui == 0):
                        pop_filler(1)
                    if kind == "pair":
                        pt8 = pt8p.tile([128, 2, 2, QW], FP8, tag="pt8")
                        score_slot(qc, p, 2 * idx, pt8=pt8, par=0)
                        if pi < 4:
                            pop_filler(1)
                        score_slot(qc, p, 2 * idx + 1, pt8=pt8, par=1)
                        pend.append(dict(kind="pair", m=idx, pt8=pt8, **common))
                    else:
                        ptd = ptp.tile([128, 2, QW], BF16, tag="pt")
                        score_slot(qc, p, 4 * qc + idx, ptd=ptd, full=full)
                        pend.append(
                            dict(kind="diag", j=idx, kp=4 * qc + idx, pt=ptd,
                                 full=full, **common)
                        )
                    if pi < 4 and ui < 2:
                        pop_filler(1)
            while pend:
                pop_filler(1)
                flush_one()
            drain_before(len(PASS_ORDER))
            for pk in PASS_ORDER:
                for fn in fq[pk]:
                    fn()
                fq[pk].clear()

    nc.finalize()
    return nc


def _maskb():
    # multiplicative causal mask: 0 where q_local < kp_local, else 1
    m = np.ones((128, 128), dtype=np.float32)
    kp = np.arange(128)[:, None]
    q = np.arange(128)[None, :]
    m[q < kp] = 0.0
    return np.repeat(m[:, None, :], 2, axis=1).astype(ml_dtypes.bfloat16)


def shard_inputs(x, Wq, Wk, Wv, Wo):
    x = np.asarray(x, dtype=np.float32)
    Wq8 = np.asarray(Wq, dtype=np.float32).astype(ml_dtypes.float8_e4m3)
    Wk8 = np.asarray(Wk, dtype=np.float32).astype(ml_dtypes.float8_e4m3)
    Wv = np.asarray(Wv, dtype=ml_dtypes.bfloat16)
    Wo = np.asarray(Wo, dtype=ml_dtypes.bfloat16)
    mb = _maskb()
    xt = [np.ascontiguousarray(x[b].T) for b in range(2)]
    xt16 = [a.astype(ml_dtypes.bfloat16) for a in xt]
    xt8 = [a.astype(ml_dtypes.float8_e4m3) for a in xt]
    in_maps = []
    for c in range(8):
        b, hg = divmod(c, 4)
        sl = slice(DL * hg, DL * (hg + 1))
        in_maps.append({
            "x8": xt8[b],
            "xt": xt16[b],
            "wq": np.ascontiguousarray(Wq8[:, sl]),
            "wk": np.ascontiguousarray(Wk8[:, sl]),
            "wv": np.ascontiguousarray(Wv[:, sl]),
            "wo": np.ascontiguousarray(Wo[sl, :]),
            "maskb": mb,
        })
    return in_maps


def run(inputs, trace=False, **kwargs):
    """Build, run on 8 cores, and return (full_output, BassKernelResults)."""
    nc = build_bass()
    bo = np.asarray(inputs["bo"], dtype=np.float32)
    in_maps = shard_inputs(**{k: v for k, v in inputs.items() if k != "bo"})
    res = run_bass_kernel_spmd(
        nc, in_maps, core_ids=list(range(8)), trace=trace, **kwargs
    )
    parts = [np.asarray(r["out"], dtype=np.float32) for r in res.results]
    out = np.empty((2, S, D), dtype=np.float32)
    for b in range(2):
        out[b] = parts[4 * b] + parts[4 * b + 1] + parts[4 * b + 2] + parts[4 * b + 3]
        out[b] += bo[None, :]
    return out, res


def kernel(x, Wq, Wk, Wv, Wo, bo):
    out, _ = run(dict(x=x, Wq=Wq, Wk=Wk, Wv=Wv, Wo=Wo, bo=bo))
    return out


# revision 8
# speedup vs baseline: 1.0992x; 1.0014x over previous
"""Multi-head causal attention (B=2, S=2048, D=1024, H=16, d=64) on 8 trn2 cores.

Sharding: core c -> batch b=c//4, head-group hg=c%4 (4 heads, 256 of 1024 dims).
Each core computes its 4 heads' attention + its partial out-projection; host
sums the 4 partials per batch and adds the bias.

Design (from the bf16 baseline at ~156us to ~145us):
- q/k projections run as fp8e4m3 DoubleRow matmuls (x and Wq/Wk quantized to
  fp8 on the host, ic-chunk pairs packed per instruction) -> half the PE time
  for those matmuls. V projection and out-projection stay bf16 (rehearsed:
  V-path fp8 noise lands directly in the output).
- AV for OFF-DIAGONAL key tiles runs as fp8 DoubleRow over key-tile pairs:
  exp writes P straight to fp8, V is re-quantized to a 96-wide padded fp8
  copy (64 dims + ones column for the softmax denominator + zero pad;
  DoubleRow stationary width must be a multiple of 32). Diagonal tiles stay
  bf16: host rehearsal showed the fp8 error of the whole AV path lives in
  short rows that only touch diagonal blocks, so off-diagonal fp8 is free
  (adds 0.000 to rel err) while diagonal fp8 would blow the error budget.
- Diagonal V tiles are padded to 96 too so every AV matmul in a pass writes
  the same [96, 512] PSUM region (keeps start/stop accumulation groups
  legal); qc==0/1-style passes order units so a full-width matmul opens and
  closes the group.
- Causal mask multiply trimmed to the 128 columns containing the triangle.
- Scheduling: per-pass ordered filler queues move deferrable PE work
  (projections for later chunks, out-projection batches) into the ACT-bound
  late passes; fillers and due AV flushes are emitted BEFORE each unit's
  score matmuls so an exp-gated score never starves ready work behind it.
  Softmax denominator reciprocal runs on [1,512] before the partition
  broadcast; vext8 fp8 re-quant copies are separate fillers scheduled into
  DVE-idle windows.
- DMA: fp8 x/w startup tiles in ic-pair granularity on the sync queue (first
  DoubleRow matmul fires after ~160KB); wv/wo/bf16-x-chunk0 on the scalar
  queue before any exp is queued; later x chunks are issued 1-2 passes ahead
  of first use, all on the sync HWDGE queue (gpsimd-issued DMAs complete
  late; scalar-queue DMAs would block exp).
"""
import sys

sys.path.insert(0, "/opt/trn_rl_repo")

import numpy as np
import ml_dtypes
import concourse.bass as bass
import concourse.mybir as mybir
from concourse import bacc
from concourse.tile import TileContext
from concourse.bass_utils import run_bass_kernel_spmd

F32 = mybir.dt.float32
BF16 = mybir.dt.bfloat16
FP8 = mybir.dt.float8e4
AF = mybir.ActivationFunctionType
OP = mybir.AluOpType
DR = mybir.MatmulPerfMode.DoubleRow

S = 2048          # sequence length
D = 1024          # model dim
HD = 64           # head dim
NHL = 4           # heads per core
DL = 256          # local out dims (NHL * HD)
NQC = 4           # q chunks of 512
QW = 512          # q chunk width
NST = 16          # seq tiles of 128
NIC = 8           # input-dim chunks of 128
NPR = 4           # ic pairs (DoubleRow)
VW = 128          # vext width: 64 dims + 64 ones columns (denominator rows)
LAG = 3           # AV trails scores by this many units


def build_bass():
    nc = bacc.Bacc("TRN2", target_bir_lowering=False, debug=False, num_devices=8)

    x8_d = nc.dram_tensor("x8", [D, S], FP8, kind="ExternalInput")
    xt_d = nc.dram_tensor("xt", [D, S], BF16, kind="ExternalInput")
    wq_d = nc.dram_tensor("wq", [D, DL], FP8, kind="ExternalInput")
    wk_d = nc.dram_tensor("wk", [D, DL], FP8, kind="ExternalInput")
    wv_d = nc.dram_tensor("wv", [D, DL], BF16, kind="ExternalInput")
    wo_d = nc.dram_tensor("wo", [DL, D], BF16, kind="ExternalInput")
    mb_d = nc.dram_tensor("maskb", [128, 2, 128], BF16, kind="ExternalInput")
    out_d = nc.dram_tensor("out", [S, D], BF16, kind="ExternalOutput")

    with TileContext(nc) as tc:
        with (
            tc.tile_pool(name="consts", bufs=1) as consts,
            tc.tile_pool(name="xtp", bufs=1) as xtp,
            tc.tile_pool(name="qk", bufs=1) as qkp,
            tc.tile_pool(name="vv", bufs=1) as vvp,
            tc.tile_pool(name="ctxn", bufs=1) as ctxnp,
            tc.tile_pool(name="ptp", bufs=4) as ptp,
            tc.tile_pool(name="pt8p", bufs=4) as pt8p,
            tc.tile_pool(name="recp", bufs=4) as recp,
            tc.tile_pool(name="rbp", bufs=4) as rbp,
            tc.tile_pool(name="outp", bufs=3) as outp,
            tc.tile_pool(name="psA", bufs=2, space="PSUM") as psA,
            tc.tile_pool(name="psS", bufs=2, space="PSUM") as psS,
            tc.tile_pool(name="psC", bufs=2, space="PSUM") as psC,
        ):
            # ---- SBUF tiles
            # fp8 q/k weights: per p-half, per ic-pair tile [128, 2, 128]
            wqh = [
                [consts.tile([128, 2, 128], FP8, tag=f"wq{p}_{r}", name=f"wq{p}_{r}") for r in range(NPR)]
                for p in range(2)
            ]
            wkh = [
                [consts.tile([128, 2, 128], FP8, tag=f"wk{p}_{r}", name=f"wk{p}_{r}") for r in range(NPR)]
                for p in range(2)
            ]
            wv = consts.tile([128, NIC, DL], BF16, tag="wv")
            wo = consts.tile([128, 2, D], BF16, tag="wo")
            maskb = consts.tile([128, 2, 128], BF16, tag="maskb")
            onesr = consts.tile([1, HD], F32, tag="onesr")
            # fp8 x: chunk 0 in ic-pair tiles (startup), chunks 1-3 whole
            x8c0 = [xtp.tile([128, 2, QW], FP8, tag=f"x8c0_{r}", name=f"x8c0_{r}") for r in range(NPR)]
            x8s = [None] + [
                xtp.tile([128, NIC, QW], FP8, tag=f"x8_{c}", name=f"x8_{c}") for c in range(1, NQC)
            ]
            xts = [xtp.tile([128, NIC, QW], BF16, tag=f"xt{c}", name=f"xt{c}") for c in range(NQC)]
            qt = qkp.tile([128, 2, S], BF16, tag="qt")
            kt = qkp.tile([128, 2, S], BF16, tag="kt")
            # bf16 V (diag AV): [128, st, h, 96] (64 dims + ones + zero pad)
            vextb = vvp.tile([128, NST, NHL, VW], BF16, tag="vextb")
            # fp8 V (off-diag DR AV): key-tile pairs [128, m, h, par, 96]
            vext8 = vvp.tile([128, NST // 2, NHL, 2, VW], FP8, tag="vext8")
            ctxn = ctxnp.tile([128, 2, S], BF16, tag="ctxn")

            def x8pair(qc, r):
                return x8c0[r] if qc == 0 else x8s[qc][:, 2 * r : 2 * r + 2, :]

            # ---- DMAs.  Startup order: wq/x8 ic-pairs interleaved on sync so
            # the first DoubleRow matmul fires after ~160KB; wk pairs on scalar.
            def wpair_dma(eng, dst, src_d, p, r):
                eng.dma_start(
                    out=dst,
                    in_=src_d.ap()[256 * r : 256 * (r + 1), 128 * p : 128 * (p + 1)]
                    .rearrange("(c p) n -> p c n", p=128),
                )

            def x8c0_dma(eng, r):
                eng.dma_start(
                    out=x8c0[r],
                    in_=x8_d.ap()[256 * r : 256 * (r + 1), 0:QW].rearrange(
                        "(c p) s -> p c s", p=128
                    ),
                )

            def x8dma(eng, c, lo, hi):
                qsl = slice(c * QW, (c + 1) * QW)
                eng.dma_start(
                    out=x8s[c][:, lo:hi, :],
                    in_=x8_d.ap()[128 * lo : 128 * hi, qsl].rearrange(
                        "(c p) s -> p c s", p=128
                    ),
                )

            def xdma(eng, c, lo, hi):
                qsl = slice(c * QW, (c + 1) * QW)
                eng.dma_start(
                    out=xts[c][:, lo:hi, :],
                    in_=xt_d.ap()[128 * lo : 128 * hi, qsl].rearrange(
                        "(c p) s -> p c s", p=128
                    ),
                )

            for r in range(NPR):
                wpair_dma(nc.sync, wqh[0][r], wq_d, 0, r)
                x8c0_dma(nc.sync, r)
                wpair_dma(nc.scalar, wkh[0][r], wk_d, 0, r)
            xdma(nc.scalar, 0, 0, 4)
            nc.scalar.dma_start(out=wv, in_=wv_d.ap().rearrange("(c p) n -> p c n", p=128))
            xdma(nc.scalar, 0, 4, 8)
            for r in range(NPR):
                wpair_dma(nc.sync, wqh[1][r], wq_d, 1, r)
                wpair_dma(nc.sync, wkh[1][r], wk_d, 1, r)
            nc.scalar.dma_start(out=maskb, in_=mb_d.ap())
            nc.vector.memset(onesr, 1.0)
            # 64 ones columns: the AV matmul replicates the softmax
            # denominator across PSUM rows 64..127, so the norm needs no
            # copy/broadcast - just a [64,512] reciprocal + multiply
            nc.vector.memset(vextb[:, :, :, HD:VW], 1.0)
            nc.gpsimd.memset(vext8[:, :, :, :, HD:VW], 1.0)

            # ---- PE filler scheduling: per-pass ordered queues + overflow.
            # Pass order interleaves the ACT-heavy qc=3 passes mid-kernel so
            # deferred projections/out-projections can feed the PE there.
            PASS_ORDER = [(0, 0), (0, 1), (1, 0), (1, 1), (2, 0), (2, 1), (3, 0), (3, 1)]
            fq = {pk: [] for pk in PASS_ORDER}
            overflow = []
            cur_pass = [0]

            def pop_filler(n=1):
                for _ in range(n):
                    q = fq[PASS_ORDER[cur_pass[0]]]
                    if q:
                        q.pop(0)()
                    elif overflow:
                        overflow.pop(0)()
                    else:
                        return

            def drain_before(pi):
                for pk in PASS_ORDER[:pi]:
                    for fn in fq[pk]:
                        fn()
                    fq[pk].clear()
                while overflow:
                    overflow.pop(0)()

            def mk_qtkt(dst, whs, p, qc):
                def go():
                    acc = psA.tile([128, QW], F32, tag="pa")
                    for r in range(NPR):
                        nc.tensor.matmul(
                            acc,
                            whs[p][r],
                            x8pair(qc, r),
                            start=(r == 0),
                            stop=(r == NPR - 1),
                            perf_mode=DR,
                        )
                    nc.vector.tensor_copy(dst[:, p, qc * QW : (qc + 1) * QW], acc)
                return go

            def mk_v(st):
                def go():
                    c, r = divmod(st, 4)
                    acc_t = psA.tile([128, QW], F32, tag="pa")
                    acc = acc_t[:, 0:DL]
                    for ic in range(NIC):
                        nc.tensor.matmul(
                            acc,
                            xts[c][:, ic, 128 * r : 128 * (r + 1)],
                            wv[:, ic, :],
                            start=(ic == 0),
                            stop=(ic == NIC - 1),
                        )
                    nc.vector.tensor_copy(
                        vextb[:, st, :, 0:HD], acc.rearrange("p (h e) -> p h e", h=NHL)
                    )
                return go

            def mk_v8cast(st):
                def go():
                    nc.vector.tensor_copy(
                        vext8[:, st // 2, :, st % 2, 0:HD], vextb[:, st, :, 0:HD]
                    )
                return go

            def mk_outproj(t, tail=False):
                def go():
                    tsl = slice(128 * t, 128 * (t + 1))
                    osb = outp.tile([128, D], BF16, tag="osb")
                    for nh in range(2):
                        po = psA.tile([128, QW], F32, tag="pa")
                        nsl = slice(QW * nh, QW * (nh + 1))
                        nc.tensor.matmul(
                            po, ctxn[:, 0, tsl], wo[:, 0, nsl], start=True, stop=False
                        )
                        nc.tensor.matmul(
                            po, ctxn[:, 1, tsl], wo[:, 1, nsl], start=False, stop=True
                        )
                        if tail and nh == 1:
                            nc.scalar.copy(osb[:, nsl], po)
                        else:
                            nc.vector.tensor_copy(osb[:, nsl], po)
                        if tail:
                            eng = nc.scalar if (t + nh) % 2 else nc.sync
                            eng.dma_start(out=out_d.ap()[tsl, nsl], in_=osb[:, nsl])
                    if not tail:
                        nc.sync.dma_start(out=out_d.ap()[tsl, :], in_=osb)
                return go

            # ---- attention pipeline over "units" (off-diag kp pairs / diag slots)
            pend = []

            def av(it):
                if it["kind"] == "pair":
                    for h, ctx_t in ((0, it["ctxa"]), (1, it["ctxb"])):
                        nc.tensor.matmul(
                            ctx_t,
                            vext8[:, it["m"], 2 * it["p"] + h, :, :],
                            it["pt8"][:, h, :, :],
                            start=it["first"],
                            stop=it["lastu"],
                            perf_mode=DR,
                        )
                elif it["full"]:
                    # qc==0: full-width diag AV (pt zero-padded) so start/stop
                    # accumulation flags always cover the whole [VW, QW] region
                    for h, ctx_t in ((0, it["ctxa"]), (1, it["ctxb"])):
                        nc.tensor.matmul(
                            ctx_t,
                            vextb[:, it["kp"], 2 * it["p"] + h, :],
                            it["pt"][:, h, :],
                            start=it["first"],
                            stop=it["lastu"],
                        )
                else:
                    j = it["j"]
                    w = QW - 128 * j
                    for h, ctx_t in ((0, it["ctxa"]), (1, it["ctxb"])):
                        nc.tensor.matmul(
                            ctx_t[:, 128 * j : QW],
                            vextb[:, it["kp"], 2 * it["p"] + h, :],
                            it["pt"][:, h, 0:w],
                            start=it["first"],
                            stop=it["lastu"],
                        )

            def norm(it):
                qc, p = it["qc"], it["p"]
                last = qc == 3 and p == 1
                qsl = slice(qc * QW, (qc + 1) * QW)
                for h, ctx_t in ((0, it["ctxa"]), (1, it["ctxb"])):
                    rb0 = rbp.tile([HD, QW], F32, tag="rb0")
                    nc.vector.tensor_copy(rb0, ctx_t[HD : 2 * HD, :])
                    rb = rbp.tile([HD, QW], F32, tag="rb")
                    nc.vector.reciprocal_approx_fast(rb, rb0)
                    nc.vector.scalar_tensor_tensor(
                        out=ctxn[64 * h : 64 * h + 64, p, qsl],
                        in0=ctx_t[0:HD, :],
                        scalar=1.0,
                        in1=rb,
                        op0=OP.mult,
                        op1=OP.mult,
                    )

            def flush_one():
                it = pend.pop(0)
                av(it)
                if it["lastu"]:
                    norm(it)
                    if it["p"] == 1:
                        tail = it["qc"] == 3
                        host = OUT_HOST[it["qc"]]
                        for ti, t in enumerate(range(4 * it["qc"], 4 * it["qc"] + 4)):
                            h2 = (3, 1) if (it["qc"] == 1 and ti >= 2) else host
                            fq[h2].append(mk_outproj(t, tail=tail))

            # ---- prelude + filler plan
            mk_qtkt(qt, wqh, 0, 0)()
            mk_qtkt(kt, wkh, 0, 0)()
            fq[(0, 0)] = [mk_v(0), mk_v(1), mk_v(2), mk_v(3),
                          mk_qtkt(qt, wqh, 1, 0), mk_qtkt(kt, wkh, 1, 0)]
            fq[(0, 1)] = [mk_qtkt(qt, wqh, 0, 1), mk_qtkt(kt, wkh, 0, 1),
                          mk_v8cast(0), mk_v8cast(1), mk_v8cast(2), mk_v8cast(3)]
            fq[(1, 0)] = [mk_v(4), mk_v(5), mk_v(6), mk_v(7),
                          mk_qtkt(qt, wqh, 1, 1), mk_qtkt(kt, wkh, 1, 1)]
            fq[(1, 1)] = [mk_qtkt(qt, wqh, 0, 2), mk_qtkt(kt, wkh, 0, 2),
                          mk_v8cast(4), mk_v8cast(5), mk_v8cast(6), mk_v8cast(7)]
            fq[(2, 0)] = [mk_v(8), mk_v(9), mk_v(10), mk_v(11),
                          mk_qtkt(qt, wqh, 1, 2), mk_qtkt(kt, wkh, 1, 2)]
            fq[(2, 1)] = [mk_qtkt(qt, wqh, 0, 3), mk_qtkt(kt, wkh, 0, 3),
                          mk_v(12), mk_v(13),
                          mk_v8cast(8), mk_v8cast(9), mk_v8cast(10), mk_v8cast(11)]
            fq[(3, 0)] = [mk_v(14), mk_v(15),
                          mk_qtkt(qt, wqh, 1, 3), mk_qtkt(kt, wkh, 1, 3)]
            OUT_HOST = {0: (3, 1), 1: (3, 0), 2: (3, 1), 3: (3, 1)}

            def score_slot(qc, p, kp, pt8=None, par=0, ptd=None, full=False):
                """Score matmuls + exp for one kp slot."""
                qsl = slice(qc * QW, (qc + 1) * QW)
                ksl = slice(kp * 128, (kp + 1) * 128)
                diag = kp >= 4 * qc
                st_t = psS.tile([128, 2, QW], F32, tag="st")
                if not diag:
                    for h in (0, 1):
                        nc.tensor.matmul(
                            st_t[:, h, :],
                            kt[64 * h : 64 * h + 64, p, ksl],
                            qt[64 * h : 64 * h + 64, p, qsl],
                            start=True, stop=True,
                            tile_position=(64 * h, 0),
                        )
                    nc.scalar.activation(pt8[:, :, par, :], st_t, AF.Exp, scale=0.125)
                else:
                    j = kp - 4 * qc
                    w = QW - 128 * j
                    qtr = slice(qc * QW + 128 * j, (qc + 1) * QW)
                    for h in (0, 1):
                        nc.tensor.matmul(
                            st_t[:, h, 0:w],
                            kt[64 * h : 64 * h + 64, p, ksl],
                            qt[64 * h : 64 * h + 64, p, qtr],
                            start=True, stop=True,
                            tile_position=(64 * h, 0),
                        )
                    off = 128 * j if full else 0
                    if full and j > 0:
                        nc.gpsimd.memset(ptd[:, :, 0 : 128 * j], 0.0)
                    nc.scalar.activation(
                        ptd[:, :, off : off + w], st_t[:, :, 0:w], AF.Exp, scale=0.125
                    )
                    mw = min(w, 128)
                    nc.vector.tensor_mul(
                        ptd[:, :, off : off + mw],
                        ptd[:, :, off : off + mw],
                        maskb[:, :, 0:mw],
                    )

            # ---- main loop over passes
            for pi, (qc, p) in enumerate(PASS_ORDER):
                cur_pass[0] = pi
                drain_before(pi)
                full = qc == 0
                ctxa = psC.tile([VW, QW], F32, tag="ctx")
                ctxb = psC.tile([VW, QW], F32, tag="ctx")
                units = [("pair", m) for m in range(2 * qc)]
                units += [("diag", j) for j in range(4)]
                if not full and qc > 0:
                    # trimmed diag mode: a full-width pair must open and close
                    # the PSUM accumulation group
                    units.append(units.pop(2 * qc - 1))
                n_units = len(units)
                for ui, (kind, idx) in enumerate(units):
                    if pi == 0 and ui == 1:
                        x8dma(nc.sync, 1, 0, 4)
                        x8dma(nc.sync, 1, 4, 8)
                        xdma(nc.sync, 1, 0, 4)
                        xdma(nc.sync, 1, 4, 8)
                        nc.sync.dma_start(
                            out=wo,
                            in_=wo_d.ap().rearrange("(c p) n -> p c n", p=128),
                        )
                    if pi == 1 and ui == 0:
                        x8dma(nc.sync, 2, 0, 4)
                        x8dma(nc.sync, 2, 4, 8)
                        xdma(nc.sync, 2, 0, 4)
                        xdma(nc.sync, 2, 4, 8)
                    if pi == 2 and ui == 0:
                        x8dma(nc.sync, 3, 0, 4)
                        x8dma(nc.sync, 3, 4, 8)
                        xdma(nc.sync, 3, 0, 4)
                        xdma(nc.sync, 3, 4, 8)
                    common = dict(
                        qc=qc, p=p, ctxa=ctxa, ctxb=ctxb,
                        first=(ui == 0), lastu=(ui == n_units - 1),
                    )
                    while len(pend) > LAG:
                        flush_one()
                    if not (pi == 0 and ui == 0):
                        pop_filler(1)
                    if kind == "pair":
                        pt8 = pt8p.tile([128, 2, 2, QW], FP8, tag="pt8")
                        score_slot(qc, p, 2 * idx, pt8=pt8, par=0)
                        if pi < 4:
                            pop_filler(1)
                        score_slot(qc, p, 2 * idx + 1, pt8=pt8, par=1)
                        pend.append(dict(kind="pair", m=idx, pt8=pt8, **common))
                    else:
                        ptd = ptp.tile([128, 2, QW], BF16, tag="pt")
                        score_slot(qc, p, 4 * qc + idx, ptd=ptd, full=full)
                        pend.append(
                            dict(kind="diag", j=idx, kp=4 * qc + idx, pt=ptd,
                                 full=full, **common)
                        )
                    if ui < 2:
                        pop_filler(1)
            while pend:
                pop_filler(1)
                flush_one()
            drain_before(len(PASS_ORDER))
            for pk in PASS_ORDER:
                for fn in fq[pk]:
                    fn()
                fq[pk].clear()

    nc.finalize()
    return nc


def _maskb():
    # multiplicative causal mask: 0 where q_local < kp_local, else 1
    m = np.ones((128, 128), dtype=np.float32)
    kp = np.arange(128)[:, None]
    q = np.arange(128)[None, :]
    m[q < kp] = 0.0
    return np.repeat(m[:, None, :], 2, axis=1).astype(ml_dtypes.bfloat16)


def shard_inputs(x, Wq, Wk, Wv, Wo):
    x = np.asarray(x, dtype=np.float32)
    Wq8 = np.asarray(Wq, dtype=np.float32).astype(ml_dtypes.float8_e4m3)
    Wk8 = np.asarray(Wk, dtype=np.float32).astype(ml_dtypes.float8_e4m3)
    Wv = np.asarray(Wv, dtype=ml_dtypes.bfloat16)
    Wo = np.asarray(Wo, dtype=ml_dtypes.bfloat16)
    mb = _maskb()
    xt = [np.ascontiguousarray(x[b].T) for b in range(2)]
    xt16 = [a.astype(ml_dtypes.bfloat16) for a in xt]
    xt8 = [a.astype(ml_dtypes.float8_e4m3) for a in xt]
    in_maps = []
    for c in range(8):
        b, hg = divmod(c, 4)
        sl = slice(DL * hg, DL * (hg + 1))
        in_maps.append({
            "x8": xt8[b],
            "xt": xt16[b],
            "wq": np.ascontiguousarray(Wq8[:, sl]),
            "wk": np.ascontiguousarray(Wk8[:, sl]),
            "wv": np.ascontiguousarray(Wv[:, sl]),
            "wo": np.ascontiguousarray(Wo[sl, :]),
            "maskb": mb,
        })
    return in_maps


def run(inputs, trace=False, **kwargs):
    """Build, run on 8 cores, and return (full_output, BassKernelResults)."""
    nc = build_bass()
    bo = np.asarray(inputs["bo"], dtype=np.float32)
    in_maps = shard_inputs(**{k: v for k, v in inputs.items() if k != "bo"})
    res = run_bass_kernel_spmd(
        nc, in_maps, core_ids=list(range(8)), trace=trace, **kwargs
    )
    parts = [np.asarray(r["out"], dtype=np.float32) for r in res.results]
    out = np.empty((2, S, D), dtype=np.float32)
    for b in range(2):
        out[b] = parts[4 * b] + parts[4 * b + 1] + parts[4 * b + 2] + parts[4 * b + 3]
        out[b] += bo[None, :]
    return out, res


def kernel(x, Wq, Wk, Wv, Wo, bo):
    out, _ = run(dict(x=x, Wq=Wq, Wk=Wk, Wv=Wv, Wo=Wo, bo=bo))
    return out
